# revision 23
# baseline (speedup 1.0000x reference)
"""Trainium2 Bass kernel for nn_IxformerQuantMoe (quantized top-2 MoE, E=8 experts).

Strategy (expert-parallel across 8 NeuronCores):
  - Host computes the fp32 gate (softmax + top-2 + renormalize), the per-token
    dynamic int8 quantization of hidden_states (exact numpy replica of the
    reference), and routes tokens: for each expert e, gathers the quantized
    rows of the tokens whose top-2 contains e, padded to a common capacity C
    (multiple of 128), pre-transposed to the contraction-major tile layout
    the PE array consumes.
  - Core e runs the expert FFN for its token set:
      int8 GEMM fc1 (exact in bf16, fp32 PSUM) -> dequant+SwiGLU -> dynamic
      requant (round-to-nearest via the 1.5*2^23 trick) -> DMA-xbar transpose
      -> int8 GEMM fc2 -> dequant + gate scaling.
    The per-tile work is software-pipelined one tile ahead so the PE array
    never waits on the dequant/requant chain.
  - Host scatter-adds each expert's output rows into the final [T, H] output
    (the weighted top-2 combine).
"""

import os
import sys

for _p in ("/opt/trn_rl_repo", "/root/.axon_site/_ro/trn_rl_repo"):
    if os.path.isdir(_p) and _p not in sys.path:
        sys.path.insert(0, _p)

import numpy as np
import ml_dtypes

import concourse.bass as bass
import concourse.bacc as bacc
import concourse.tile as tile
from concourse import mybir
from concourse.bass import ds, ts
from concourse.bass_utils import run_bass_kernel_spmd

T, H, I, E, TOPK = 4096, 2048, 1408, 8, 2
KT1 = H // 128     # 16 k-tiles for fc1 contraction
KT2 = I // 128     # 11 k-tiles for fc2 contraction
TWO23 = 12582912.0  # 1.5*2^23: fp32 add/sub rounds to nearest integer (RNE)

F32 = mybir.dt.float32
BF16 = mybir.dt.bfloat16
I8 = mybir.dt.int8

FC1_GROUPS = [(0, 512), (512, 512), (1024, 384)]   # column groups over I
RQ_CHUNKS = [(0, 512), (512, 512), (1024, 384)]    # requant chunks over I

_cache = {}
LAST_EXEC_NS = None


def _build_program(C):
    """Bass program run identically (SPMD) on 8 cores; per-core data differs."""
    nt = C // 128
    nc = bacc.Bacc(None, target_bir_lowering=False)

    qt_d = nc.declare_dram_parameter("qt", [nt, 128, KT1, 128], BF16, isOutput=False)
    sin_d = nc.declare_dram_parameter("sin", [128, nt], F32, isOutput=False)
    r_d = nc.declare_dram_parameter("r", [128, nt], F32, isOutput=False)
    w13_d = nc.declare_dram_parameter("w13t", [KT1, 128, 2 * I], I8, isOutput=False)
    w2_d = nc.declare_dram_parameter("w2t", [KT2, 128, H], I8, isOutput=False)
    s13_d = nc.declare_dram_parameter("s13", [2 * I], F32, isOutput=False)
    s2w_d = nc.declare_dram_parameter("s2w", [H], F32, isOutput=False)
    y_d = nc.declare_dram_parameter("y", [C, H], F32, isOutput=True)

    with tile.TileContext(nc) as tc:
        with (
            tc.tile_pool(name="singles", bufs=1) as singles,
            tc.tile_pool(name="qtp", bufs=2) as qtp,
            tc.tile_pool(name="gp", bufs=2) as gp,
            tc.tile_pool(name="actp", bufs=2) as actp,
            tc.tile_pool(name="aqp", bufs=2) as aqp,
            tc.tile_pool(name="rqp", bufs=2) as rqp,
            tc.tile_pool(name="qatp", bufs=2) as qatp,
            tc.tile_pool(name="yp", bufs=2) as yp,
            tc.tile_pool(name="wsp", bufs=2) as wsp,
            tc.tile_pool(name="sp", bufs=3) as sp,
            tc.tile_pool(name="ps1", bufs=2, space="PSUM") as ps1,
            tc.tile_pool(name="ps2", bufs=2, space="PSUM") as ps2,
        ):
            # ---- resident tiles ----
            w13_sb = singles.tile([128, KT1, 2 * I], BF16)
            w2_sb = singles.tile([128, KT2, H], BF16)
            s13g_b = singles.tile([128, I], F32)
            s13u_b = singles.tile([128, I], F32)
            s2w_b = singles.tile([128, H], F32)
            sin_sb = singles.tile([128, nt], F32)
            r_sb = singles.tile([128, nt], F32)

            neg223 = singles.tile([128, 1], F32)
            nc.vector.memset(neg223, -TWO23)
            pos223 = singles.tile([128, 1], F32)
            nc.vector.memset(pos223, TWO23)
            zero_b = singles.tile([128, 1], F32)
            nc.vector.memset(zero_b, 0.0)

            qts = {}

            def prep(t):
                """DMA the pre-transposed quantized input tile for token-tile t."""
                qt = qtp.tile([128, KT1, 128], BF16, tag="qt")
                nc.sync.dma_start(qt, x_like(qt_d)[t])
                qts[t] = qt

            # ---- prologue DMA order: int8 weights streamed and up-converted
            # on-chip (w13 alternating DVE/Act, w2 on GpSimd) so fc1(0) can
            # start as soon as w13 k-tile 0 lands.
            for k in range(KT1):
                ws = wsp.tile([128, 2 * I], I8, tag=("wse" if k % 2 == 0 else "wso"))
                if k == 0:
                    # split k0 so the braid's g-side matmuls can start before
                    # the u-side columns finish converting
                    nc.sync.dma_start(ws[:, :I], x_like(w13_d)[k][:, :I])
                    nc.vector.tensor_copy(w13_sb[:, k, :I], ws[:, :I])
                    prep(0)
                    nc.sync.dma_start(ws[:, I:], x_like(w13_d)[k][:, I:])
                    nc.vector.tensor_copy(w13_sb[:, k, I:], ws[:, I:])
                    continue
                nc.sync.dma_start(ws, x_like(w13_d)[k])
                if k % 3 != 2:
                    nc.vector.tensor_copy(w13_sb[:, k, :], ws)
                else:
                    nc.gpsimd.tensor_copy(w13_sb[:, k, :], ws)
                if k == 3 and nt > 1:
                    prep(1)
            nc.sync.dma_start(sin_sb, x_like(sin_d))
            nc.sync.dma_start(r_sb, x_like(r_d))
            nc.sync.dma_start(s13g_b, _bcast128(s13_d, 0, I))
            nc.sync.dma_start(s13u_b, _bcast128(s13_d, I, I))
            if nt > 2:
                prep(2)
            for k in range(KT2):
                ws = wsp.tile([128, 2 * I], I8, tag=("wse" if k % 2 == 0 else "wso"))
                nc.sync.dma_start(ws[:, :H], x_like(w2_d)[k])
                nc.gpsimd.tensor_copy(w2_sb[:, k, :], ws[:, :H])
            nc.sync.dma_start(s2w_b, _bcast128(s2w_d, 0, H))

            def fc1(t):
                """PE: int8-as-bf16 GEMM into PSUM group pairs."""
                pgus = []
                for off, cw in FC1_GROUPS:
                    pg = ps1.tile([128, 512], F32, tag="psg")
                    pu = ps1.tile([128, 512], F32, tag="psu")
                    for k in range(KT1):
                        nc.tensor.matmul(
                            pg[:, :cw], qts[t][:, k, :], w13_sb[:, k, ds(off, cw)],
                            start=(k == 0), stop=(k == KT1 - 1),
                        )
                        nc.tensor.matmul(
                            pu[:, :cw], qts[t][:, k, :], w13_sb[:, k, ds(I + off, cw)],
                            start=(k == 0), stop=(k == KT1 - 1),
                        )
                    pgus.append((off, cw, pg, pu))
                del qts[t]
                return pgus

            def fc1_braid():
                """Prologue fc1 for tiles 0 and 1-group0, k-outer so the PE
                consumes w13 k-tiles in DMA arrival order (all 8 PSUM banks)."""
                t0g = []
                for gi, (off, cw) in enumerate(FC1_GROUPS):
                    pg = (ps1 if gi < 2 else ps2).tile(
                        [128, 512], F32, tag=("psg" if gi < 2 else "psa"))
                    pu = (ps1 if gi < 2 else ps2).tile(
                        [128, 512], F32, tag=("psu" if gi < 2 else "psb"))
                    t0g.append((off, cw, pg, pu))
                pgB = ps2.tile([128, 512], F32, tag="psa")
                puB = ps2.tile([128, 512], F32, tag="psb")
                # tile 1's k-sequence is rotated so its first matmuls don't
                # wait on the qt(1) DMA that lands after w13 k-tile 3
                t1ks = list(range(6, KT1)) + list(range(6))
                for ki in range(KT1):
                    k = ki
                    for off, cw, pg, pu in t0g:
                        nc.tensor.matmul(
                            pg[:, :cw], qts[0][:, k, :], w13_sb[:, k, ds(off, cw)],
                            start=(k == 0), stop=(k == KT1 - 1),
                        )
                        nc.tensor.matmul(
                            pu[:, :cw], qts[0][:, k, :], w13_sb[:, k, ds(I + off, cw)],
                            start=(k == 0), stop=(k == KT1 - 1),
                        )
                    kb = t1ks[ki]
                    off, cw = FC1_GROUPS[0]
                    nc.tensor.matmul(
                        pgB[:, :cw], qts[1][:, kb, :], w13_sb[:, kb, ds(off, cw)],
                        start=(ki == 0), stop=(ki == KT1 - 1),
                    )
                    nc.tensor.matmul(
                        puB[:, :cw], qts[1][:, kb, :], w13_sb[:, kb, ds(I + off, cw)],
                        start=(ki == 0), stop=(ki == KT1 - 1),
                    )
                del qts[0]
                return t0g, (FC1_GROUPS[0][0], FC1_GROUPS[0][1], pgB, puB)

            def fc1_rest1(g0pair):
                """Tile 1 groups 1..2 (group-inner), after the braid."""
                pgus = [g0pair]
                for off, cw in FC1_GROUPS[1:]:
                    pg = ps1.tile([128, 512], F32, tag="psg")
                    pu = ps1.tile([128, 512], F32, tag="psu")
                    for k in range(KT1):
                        nc.tensor.matmul(
                            pg[:, :cw], qts[1][:, k, :], w13_sb[:, k, ds(off, cw)],
                            start=(k == 0), stop=(k == KT1 - 1),
                        )
                        nc.tensor.matmul(
                            pu[:, :cw], qts[1][:, k, :], w13_sb[:, k, ds(I + off, cw)],
                            start=(k == 0), stop=(k == KT1 - 1),
                        )
                    pgus.append((off, cw, pg, pu))
                del qts[1]
                return pgus

            def chain(t, pgus):
                """DVE/Act: dequant + SwiGLU + abs-max + requant scales.

                g = (pg * s_in) * s13g ; u = (pu * s_in) * s13u  (reference
                association); act = silu(g) * u; s_a = max|act|/127.
                The s_in tensor_scalar comes first so the PSUM bank frees
                without waiting on the s13 broadcast DMAs.
                """
                sin_col = sin_sb[:, t : t + 1]
                act_t = actp.tile([128, I], F32, tag="act")
                mxs = []
                for gi, (off, cw, pg, pu) in enumerate(pgus):
                    g_sc = gp.tile([128, 512], F32, tag="gsc")
                    nc.vector.tensor_scalar_mul(g_sc[:, :cw], pg[:, :cw], sin_col)
                    nc.vector.tensor_tensor(
                        g_sc[:, :cw], g_sc[:, :cw], s13g_b[:, ds(off, cw)],
                        mybir.AluOpType.mult,
                    )
                    nc.scalar.activation(
                        g_sc[:, :cw], g_sc[:, :cw],
                        mybir.ActivationFunctionType.Silu, bias=zero_b, scale=1.0,
                    )
                    nc.vector.tensor_scalar_mul(
                        act_t[:, ds(off, cw)], pu[:, :cw], sin_col
                    )
                    nc.vector.tensor_tensor(
                        act_t[:, ds(off, cw)], act_t[:, ds(off, cw)],
                        s13u_b[:, ds(off, cw)], mybir.AluOpType.mult,
                    )
                    nc.vector.tensor_tensor(
                        act_t[:, ds(off, cw)], act_t[:, ds(off, cw)], g_sc[:, :cw],
                        mybir.AluOpType.mult,
                    )
                    mx = sp.tile([128, 1], F32, tag=f"mx{gi}")
                    nc.vector.tensor_reduce(
                        mx, act_t[:, ds(off, cw)], axis=mybir.AxisListType.X,
                        op=mybir.AluOpType.max, apply_absolute_value=True,
                    )
                    mxs.append(mx)
                m2 = sp.tile([128, 1], F32, tag="m2")
                nc.vector.tensor_tensor(m2, mxs[0], mxs[1], mybir.AluOpType.max)
                nc.vector.tensor_tensor(m2, m2, mxs[2], mybir.AluOpType.max)
                s_tr = sp.tile([128, 1], F32, tag="s_tr")
                nc.vector.tensor_scalar(
                    s_tr, m2, 1.0 / 127.0, 1e-8,
                    mybir.AluOpType.mult, mybir.AluOpType.max,
                )
                sc_eff = sp.tile([128, 1], F32, tag="sc_eff")  # 1 / s_a
                nc.vector.reciprocal(sc_eff, s_tr)
                c_t = sp.tile([128, 1], F32, tag="c")  # final row scale r * s_a
                nc.vector.tensor_tensor(
                    c_t, s_tr, r_sb[:, t : t + 1], mybir.AluOpType.mult
                )
                return act_t, sc_eff, c_t

            def requant(t, act_t, sc_eff):
                """Act: round(act' * sc_eff) -> bf16 ints; DMA-xbar transpose."""
                act_q = aqp.tile([128, I], BF16, tag="aq")
                for off, cw in RQ_CHUNKS:
                    t1 = rqp.tile([128, 512], F32, tag="rq")
                    nc.scalar.activation(
                        t1[:, :cw], act_t[:, ds(off, cw)],
                        mybir.ActivationFunctionType.Identity,
                        bias=pos223, scale=sc_eff,
                    )
                    nc.scalar.activation(
                        act_q[:, ds(off, cw)], t1[:, :cw],
                        mybir.ActivationFunctionType.Identity, bias=neg223,
                    )
                qa = qatp.tile([128, KT2, 128], BF16, tag="qa")
                nc.sync.dma_start_transpose(qa[:], act_q[:])
                return qa

            def fc2(t, qa):
                """PE: second int8 GEMM."""
                ppairs = []
                for op_ in range(2):
                    pa = ps2.tile([128, 512], F32, tag="psa")
                    pb = ps2.tile([128, 512], F32, tag="psb")
                    for k in range(KT2):
                        nc.tensor.matmul(
                            pa, qa[:, k, :], w2_sb[:, k, ts(2 * op_, 512)],
                            start=(k == 0), stop=(k == KT2 - 1),
                        )
                        nc.tensor.matmul(
                            pb, qa[:, k, :], w2_sb[:, k, ts(2 * op_ + 1, 512)],
                            start=(k == 0), stop=(k == KT2 - 1),
                        )
                    ppairs.append((2 * op_, pa))
                    ppairs.append((2 * op_ + 1, pb))
                return ppairs

            def store(t, ppairs, c_t):
                """DVE: dequant fc2 + per-token gate scale; DMA out."""
                t0 = t * 128
                for oc_i, pp in ppairs:
                    oc = oc_i * 512
                    yc = yp.tile([128, 512], F32, tag="yc")
                    nc.vector.tensor_tensor(
                        yc, pp, s2w_b[:, ds(oc, 512)], mybir.AluOpType.mult
                    )
                    nc.vector.tensor_scalar_mul(yc, yc, c_t[:, :])
                    nc.sync.dma_start(x_like(y_d)[t0 : t0 + 128, ds(oc, 512)], yc)

            # ---- software-pipelined tile loop ----
            state = {}
            if nt >= 3:
                # Prologue: braided fc1(0)+fc1(1).g0 (k-outer, all 8 PSUM
                # banks) rides the w13 DMA stream; fc1(2) is front-loaded so
                # the PE stays busy while the w2 stream lands before fc2(0).
                t0g, g0pair = fc1_braid()
                pgus1 = fc1_rest1(g0pair)
                state[0] = chain(0, t0g)
                state[1] = chain(1, pgus1)
                pgus2 = fc1(2)
                act_t, sc_eff, c_t = state.pop(0)
                qa = requant(0, act_t, sc_eff)
                ppairs = fc2(0, qa)
                state[2] = chain(2, pgus2)
                store(0, ppairs, c_t)
                if 3 < nt:
                    prep(3)
                for t in range(1, nt):
                    if t + 2 < nt:
                        pgus_n = fc1(t + 2)
                    act_t, sc_eff, c_t = state.pop(t)
                    qa = requant(t, act_t, sc_eff)
                    ppairs = fc2(t, qa)
                    if t + 2 < nt:
                        state[t + 2] = chain(t + 2, pgus_n)
                    store(t, ppairs, c_t)
                    if t + 3 < nt:
                        prep(t + 3)
            else:
                pgus = fc1(0)
                state[0] = chain(0, pgus)
                for t in range(nt):
                    if t + 1 < nt:
                        pgus_n = fc1(t + 1)
                    act_t, sc_eff, c_t = state.pop(t)
                    qa = requant(t, act_t, sc_eff)
                    ppairs = fc2(t, qa)
                    if t + 1 < nt:
                        state[t + 1] = chain(t + 1, pgus_n)
                    store(t, ppairs, c_t)
                    if t + 2 < nt:
                        prep(t + 2)

    nc.finalize()
    return nc


def x_like(handle):
    """DRamTensorHandle -> AP covering the whole tensor."""
    return handle[:]


def _bcast128(handle, off, n):
    """AP reading handle[off:off+n] replicated across 128 partitions."""
    ap = handle[:][ds(off, n)]
    return bass.AP(tensor=ap.tensor, offset=ap.offset, ap=[[0, 128]] + list(ap.ap))


def _host_prep(hidden_states, gate_weight, w13_weight, w13_weight_scale,
               w2_weight, w2_weight_scale):
    """Host routing + quantization + per-core input maps. -> (in_maps, idxs, C)."""
    x = np.ascontiguousarray(np.asarray(hidden_states, dtype=np.float32))
    gw = np.asarray(gate_weight, dtype=np.float32)
    w13 = np.asarray(w13_weight)
    s13 = np.ascontiguousarray(np.asarray(w13_weight_scale, dtype=np.float32))
    w2 = np.asarray(w2_weight)
    s2w = np.ascontiguousarray(np.asarray(w2_weight_scale, dtype=np.float32))

    # ---- host routing: fp32 gate, softmax, top-2, renormalize ----
    logits = (x @ gw.T).astype(np.float32)
    p = np.exp(logits - logits.max(axis=1, keepdims=True), dtype=np.float32)
    p = (p / p.sum(axis=1, keepdims=True)).astype(np.float32)
    topi = np.argsort(-p, axis=1, kind="stable")[:, :TOPK]  # ties -> lower index
    topv = np.take_along_axis(p, topi, axis=1).astype(np.float32)
    gates = (topv / topv.sum(axis=1, keepdims=True)).astype(np.float32)

    # ---- host dynamic per-token int8 quant (exact reference replica) ----
    s_in = np.maximum(np.abs(x).max(axis=1, keepdims=True) / 127.0, 1e-8)
    s_in = s_in.astype(np.float32)
    q = np.clip(np.round(x / s_in), -127.0, 127.0).astype(np.float32)

    idxs, rvals = [], []
    for e in range(E):
        sel = topi == e
        tok = np.nonzero(sel.any(axis=1))[0]
        r = (gates * sel)[tok].sum(axis=1).astype(np.float32)
        idxs.append(tok)
        rvals.append(r)

    cap = max(128, max(len(t) for t in idxs))
    C = ((cap + 127) // 128) * 128
    nt = C // 128

    q_bf = q.astype(ml_dtypes.bfloat16)
    in_maps = []
    for e in range(E):
        n_e = len(idxs[e])
        qg = np.zeros((C, H), dtype=ml_dtypes.bfloat16)
        qg[:n_e] = q_bf[idxs[e]]
        # qt[t, p, k, j] = qg[t*128 + j, 128*k + p]
        qt = np.ascontiguousarray(
            qg.T.reshape(KT1, 128, nt, 128).transpose(2, 1, 0, 3)
        )
        sin_flat = np.zeros(C, dtype=np.float32)
        sin_flat[:n_e] = s_in[idxs[e], 0]
        sin_pack = np.ascontiguousarray(sin_flat.reshape(nt, 128).T)
        r_flat = np.zeros(C, dtype=np.float32)
        r_flat[:n_e] = rvals[e]
        r_pack = np.ascontiguousarray(r_flat.reshape(nt, 128).T)
        w13t = np.ascontiguousarray(w13[e].T).reshape(KT1, 128, 2 * I)
        w2t = np.ascontiguousarray(w2[e].T).reshape(KT2, 128, H)
        in_maps.append({
            "qt": qt,
            "sin": sin_pack,
            "r": r_pack,
            "w13t": w13t,
            "w2t": w2t,
            "s13": np.ascontiguousarray(s13[e]),
            "s2w": np.ascontiguousarray(s2w[e]),
        })
    return in_maps, idxs, C


def kernel(hidden_states, gate_weight, w13_weight, w13_weight_scale,
           w2_weight, w2_weight_scale):
    in_maps, idxs, C = _host_prep(hidden_states, gate_weight, w13_weight,
                                  w13_weight_scale, w2_weight, w2_weight_scale)
    if C not in _cache:
        _cache[C] = _build_program(C)
    nc = _cache[C]

    trace = bool(int(os.environ.get("MOE_TRACE", "0")))
    br = run_bass_kernel_spmd(nc, in_maps, list(range(E)), trace=trace)
    global LAST_EXEC_NS
    LAST_EXEC_NS = br.exec_time_ns
    res = br.results

    out = np.zeros((T, H), dtype=np.float32)
    for e in range(E):
        n_e = len(idxs[e])
        if n_e:
            out[idxs[e]] += np.asarray(res[e]["y"])[:n_e]
    return out


# revision 36
# speedup vs baseline: 1.0064x; 1.0064x over previous
"""Trainium2 Bass kernel for nn_IxformerQuantMoe (quantized top-2 MoE, E=8 experts).

Strategy (expert-parallel across 8 NeuronCores):
  - Host computes the fp32 gate (softmax + top-2 + renormalize), the per-token
    dynamic int8 quantization of hidden_states (exact numpy replica of the
    reference), and routes tokens: for each expert e, gathers the quantized
    rows of the tokens whose top-2 contains e, padded to a common capacity C
    (multiple of 128), pre-transposed to the contraction-major tile layout
    the PE array consumes.
  - Core e runs the expert FFN for its token set:
      int8 GEMM fc1 (exact in bf16, fp32 PSUM) -> dequant+SwiGLU -> dynamic
      requant (round-to-nearest via the 1.5*2^23 trick) -> DMA-xbar transpose
      -> int8 GEMM fc2 -> dequant + gate scaling.
  - Throughput structure (CoreSim: PE ~95% busy):
      * per-tile work is software-pipelined one tile ahead (fc1 of tile t+1
        runs while tile t requants/fc2s) so the PE never waits on the
        dequant/requant chain;
      * weights ship as int8 (half the HBM bytes) and are up-converted to
        bf16 on-chip on the otherwise-idle DVE/GpSimd engines;
      * the prologue runs fc1(0) k-OUTER braided with fc1(1).group0 across
        all 8 PSUM banks so the PE consumes w13 k-tiles in DMA arrival
        order, and fc1(2) is front-loaded before fc2(0) to cover the w2
        stream;
      * the dequant applies (psum * s_in) first (reference association),
        which also frees PSUM banks without waiting on the s13 scale DMAs.
  - Host scatter-adds each expert's output rows into the final [T, H] output
    (the weighted top-2 combine).
"""

import os
import sys

for _p in ("/opt/trn_rl_repo", "/root/.axon_site/_ro/trn_rl_repo"):
    if os.path.isdir(_p) and _p not in sys.path:
        sys.path.insert(0, _p)

import numpy as np
import ml_dtypes

import concourse.bass as bass
import concourse.bacc as bacc
import concourse.tile as tile
from concourse import mybir
from concourse.bass import ds, ts
from concourse.bass_utils import run_bass_kernel_spmd

T, H, I, E, TOPK = 4096, 2048, 1408, 8, 2
KT1 = H // 128     # 16 k-tiles for fc1 contraction
KT2 = I // 128     # 11 k-tiles for fc2 contraction
TWO23 = 12582912.0  # 1.5*2^23: fp32 add/sub rounds to nearest integer (RNE)

F32 = mybir.dt.float32
BF16 = mybir.dt.bfloat16
I8 = mybir.dt.int8

FC1_GROUPS = [(0, 512), (512, 512), (1024, 384)]   # column groups over I
RQ_CHUNKS = [(0, 512), (512, 512), (1024, 384)]    # requant chunks over I

_cache = {}
LAST_EXEC_NS = None


def _build_program(C):
    """Bass program run identically (SPMD) on 8 cores; per-core data differs."""
    nt = C // 128
    nc = bacc.Bacc(None, target_bir_lowering=False)

    qt_d = nc.declare_dram_parameter("qt", [nt, 128, KT1, 128], BF16, isOutput=False)
    sin_d = nc.declare_dram_parameter("sin", [128, nt], F32, isOutput=False)
    r_d = nc.declare_dram_parameter("r", [128, nt], F32, isOutput=False)
    w13_d = nc.declare_dram_parameter("w13t", [KT1, 128, 2 * I], I8, isOutput=False)
    w2_d = nc.declare_dram_parameter("w2t", [KT2, 128, H], I8, isOutput=False)
    s13_d = nc.declare_dram_parameter("s13", [2 * I], F32, isOutput=False)
    s2w_d = nc.declare_dram_parameter("s2w", [H], F32, isOutput=False)
    y_d = nc.declare_dram_parameter("y", [C, H], F32, isOutput=True)

    with tile.TileContext(nc) as tc:
        with (
            tc.tile_pool(name="singles", bufs=1) as singles,
            tc.tile_pool(name="qtp", bufs=2) as qtp,
            tc.tile_pool(name="gp", bufs=2) as gp,
            tc.tile_pool(name="actp", bufs=2) as actp,
            tc.tile_pool(name="aqp", bufs=2) as aqp,
            tc.tile_pool(name="rqp", bufs=2) as rqp,
            tc.tile_pool(name="qatp", bufs=2) as qatp,
            tc.tile_pool(name="yp", bufs=2) as yp,
            tc.tile_pool(name="wsp", bufs=2) as wsp,
            tc.tile_pool(name="sp", bufs=3) as sp,
            tc.tile_pool(name="ps1", bufs=2, space="PSUM") as ps1,
            tc.tile_pool(name="ps2", bufs=2, space="PSUM") as ps2,
        ):
            # ---- resident tiles ----
            w13_sb = singles.tile([128, KT1, 2 * I], BF16)
            w2_sb = singles.tile([128, KT2, H], BF16)
            s13g_b = singles.tile([128, I], F32)
            s13u_b = singles.tile([128, I], F32)
            s2w_b = singles.tile([128, H], F32)
            sin_sb = singles.tile([128, nt], F32)
            r_sb = singles.tile([128, nt], F32)

            neg223 = singles.tile([128, 1], F32)
            nc.vector.memset(neg223, -TWO23)
            pos223 = singles.tile([128, 1], F32)
            nc.vector.memset(pos223, TWO23)
            zero_b = singles.tile([128, 1], F32)
            nc.vector.memset(zero_b, 0.0)

            qts = {}

            def prep(t, split=False):
                """DMA the pre-transposed quantized input tile for token-tile t."""
                qt = qtp.tile([128, KT1, 128], BF16, tag="qt")
                if split:
                    # two halves so the braid's k=0 matmuls start sooner
                    half = KT1 // 2
                    nc.sync.dma_start(qt[:, :half, :], x_like(qt_d)[t][:, :half, :])
                    nc.sync.dma_start(qt[:, half:, :], x_like(qt_d)[t][:, half:, :])
                else:
                    nc.sync.dma_start(qt, x_like(qt_d)[t])
                qts[t] = qt

            # ---- prologue DMA order: int8 weights streamed and up-converted
            # on-chip (w13 alternating DVE/Act, w2 on GpSimd) so fc1(0) can
            # start as soon as w13 k-tile 0 lands.
            # warm the PE p-state/HAM during the initial DMA wait with dummy
            # matmuls on (junk) SBUF; the result is never read
            wmup = ps2.tile([128, 512], F32, tag="psa")
            for _ in range(6):
                nc.tensor.matmul(
                    wmup[:1, :], w13_sb[:, 0, 0:1], w13_sb[:, 1, 0:512],
                    start=True, stop=True,
                )

            # k0 specially split (and interleaved with qt(0)'s halves) so the
            # braid's first g-side matmuls start as early as possible
            qt0 = qtp.tile([128, KT1, 128], BF16, tag="qt")
            qts[0] = qt0
            nc.sync.dma_start(qt0[:, : KT1 // 2, :], x_like(qt_d)[0][:, : KT1 // 2, :])
            ws0 = wsp.tile([128, 2 * I], I8, tag="wse")
            nc.sync.dma_start(ws0[:, :I], x_like(w13_d)[0][:, :I])
            nc.vector.tensor_copy(w13_sb[:, 0, :I], ws0[:, :I])
            nc.sync.dma_start(ws0[:, I:], x_like(w13_d)[0][:, I:])
            nc.vector.tensor_copy(w13_sb[:, 0, I:], ws0[:, I:])
            nc.sync.dma_start(qt0[:, KT1 // 2 :, :], x_like(qt_d)[0][:, KT1 // 2 :, :])
            for k in range(1, KT1):
                ws = wsp.tile([128, 2 * I], I8, tag=("wse" if k % 2 == 0 else "wso"))
                nc.sync.dma_start(ws, x_like(w13_d)[k])
                # the braid-critical early k-tiles convert on the faster DVE
                # (g/u halves, since the braid's pg matmuls only need g);
                # later even tiles go to GpSimd, which has more slack
                if k <= 3:
                    nc.vector.tensor_copy(w13_sb[:, k, :I], ws[:, :I])
                    nc.vector.tensor_copy(w13_sb[:, k, I:], ws[:, I:])
                elif k % 2 == 0:
                    nc.gpsimd.tensor_copy(w13_sb[:, k, :], ws)
                else:
                    nc.vector.tensor_copy(w13_sb[:, k, :], ws)
                if k == 3 and nt > 1:
                    prep(1)
            nc.sync.dma_start(sin_sb, x_like(sin_d))
            nc.sync.dma_start(r_sb, x_like(r_d))
            nc.sync.dma_start(s13g_b, _bcast128(s13_d, 0, I))
            nc.sync.dma_start(s13u_b, _bcast128(s13_d, I, I))
            if nt > 2:
                prep(2)
            for k in range(KT2):
                ws = wsp.tile([128, 2 * I], I8, tag=("wse" if k % 2 == 0 else "wso"))
                nc.sync.dma_start(ws[:, :H], x_like(w2_d)[k])
                nc.gpsimd.tensor_copy(w2_sb[:, k, :], ws[:, :H])
            nc.sync.dma_start(s2w_b, _bcast128(s2w_d, 0, H))

            def fc1(t):
                """PE: int8-as-bf16 GEMM into PSUM group pairs."""
                pgus = []
                for off, cw in FC1_GROUPS:
                    pg = ps1.tile([128, 512], F32, tag="psg")
                    pu = ps1.tile([128, 512], F32, tag="psu")
                    for k in range(KT1):
                        nc.tensor.matmul(
                            pg[:, :cw], qts[t][:, k, :], w13_sb[:, k, ds(off, cw)],
                            start=(k == 0), stop=(k == KT1 - 1),
                        )
                        nc.tensor.matmul(
                            pu[:, :cw], qts[t][:, k, :], w13_sb[:, k, ds(I + off, cw)],
                            start=(k == 0), stop=(k == KT1 - 1),
                        )
                    pgus.append((off, cw, pg, pu))
                del qts[t]
                return pgus

            def fc1_braid():
                """Prologue fc1 for tiles 0 and 1-group0, k-outer so the PE
                consumes w13 k-tiles in DMA arrival order (all 8 PSUM banks)."""
                t0g = []
                for gi, (off, cw) in enumerate(FC1_GROUPS):
                    pg = (ps1 if gi < 2 else ps2).tile(
                        [128, 512], F32, tag=("psg" if gi < 2 else "psa"))
                    pu = (ps1 if gi < 2 else ps2).tile(
                        [128, 512], F32, tag=("psu" if gi < 2 else "psb"))
                    t0g.append((off, cw, pg, pu))
                pgB = ps2.tile([128, 512], F32, tag="psa")
                puB = ps2.tile([128, 512], F32, tag="psb")
                # tile 1's matmuls join the braid 3 steps late (its qt DMA
                # lands after w13 k-tile 3); the last k-tiles finish in rest1
                T1_LAG = 3
                for ki in range(KT1):
                    k = ki
                    for off, cw, pg, pu in t0g:
                        nc.tensor.matmul(
                            pg[:, :cw], qts[0][:, k, :], w13_sb[:, k, ds(off, cw)],
                            start=(k == 0), stop=(k == KT1 - 1),
                        )
                        nc.tensor.matmul(
                            pu[:, :cw], qts[0][:, k, :], w13_sb[:, k, ds(I + off, cw)],
                            start=(k == 0), stop=(k == KT1 - 1),
                        )
                    if ki < T1_LAG:
                        continue
                    kb = ki - T1_LAG
                    off, cw = FC1_GROUPS[0]
                    nc.tensor.matmul(
                        pgB[:, :cw], qts[1][:, kb, :], w13_sb[:, kb, ds(off, cw)],
                        start=(kb == 0), stop=False,
                    )
                    nc.tensor.matmul(
                        puB[:, :cw], qts[1][:, kb, :], w13_sb[:, kb, ds(I + off, cw)],
                        start=(kb == 0), stop=False,
                    )
                del qts[0]
                return t0g, (FC1_GROUPS[0][0], FC1_GROUPS[0][1], pgB, puB), T1_LAG

            def fc1_rest1(g0pair, t1_lag):
                """Tile 1: finish group 0's lagged k-tiles, then groups 1..2."""
                off, cw, pgB, puB = g0pair
                for kb in range(KT1 - t1_lag, KT1):
                    nc.tensor.matmul(
                        pgB[:, :cw], qts[1][:, kb, :], w13_sb[:, kb, ds(off, cw)],
                        start=False, stop=(kb == KT1 - 1),
                    )
                    nc.tensor.matmul(
                        puB[:, :cw], qts[1][:, kb, :], w13_sb[:, kb, ds(I + off, cw)],
                        start=False, stop=(kb == KT1 - 1),
                    )
                pgus = [g0pair]
                for off, cw in FC1_GROUPS[1:]:
                    pg = ps1.tile([128, 512], F32, tag="psg")
                    pu = ps1.tile([128, 512], F32, tag="psu")
                    for k in range(KT1):
                        nc.tensor.matmul(
                            pg[:, :cw], qts[1][:, k, :], w13_sb[:, k, ds(off, cw)],
                            start=(k == 0), stop=(k == KT1 - 1),
                        )
                        nc.tensor.matmul(
                            pu[:, :cw], qts[1][:, k, :], w13_sb[:, k, ds(I + off, cw)],
                            start=(k == 0), stop=(k == KT1 - 1),
                        )
                    pgus.append((off, cw, pg, pu))
                del qts[1]
                return pgus

            def chain(t, pgus):
                """DVE/Act: dequant + SwiGLU + abs-max + requant scales.

                g = (pg * s_in) * s13g ; u = (pu * s_in) * s13u  (reference
                association); act = silu(g) * u; s_a = max|act|/127.
                The s_in tensor_scalar comes first so the PSUM bank frees
                without waiting on the s13 broadcast DMAs.
                """
                sin_col = sin_sb[:, t : t + 1]
                act_t = actp.tile([128, I], F32, tag="act")
                mxs = []
                for gi, (off, cw, pg, pu) in enumerate(pgus):
                    g_sc = gp.tile([128, 512], F32, tag="gsc")
                    nc.vector.tensor_scalar_mul(g_sc[:, :cw], pg[:, :cw], sin_col)
                    nc.vector.tensor_tensor(
                        g_sc[:, :cw], g_sc[:, :cw], s13g_b[:, ds(off, cw)],
                        mybir.AluOpType.mult,
                    )
                    nc.scalar.activation(
                        g_sc[:, :cw], g_sc[:, :cw],
                        mybir.ActivationFunctionType.Silu, bias=zero_b, scale=1.0,
                    )
                    nc.vector.tensor_scalar_mul(
                        act_t[:, ds(off, cw)], pu[:, :cw], sin_col
                    )
                    nc.vector.tensor_tensor(
                        act_t[:, ds(off, cw)], act_t[:, ds(off, cw)],
                        s13u_b[:, ds(off, cw)], mybir.AluOpType.mult,
                    )
                    nc.vector.tensor_tensor(
                        act_t[:, ds(off, cw)], act_t[:, ds(off, cw)], g_sc[:, :cw],
                        mybir.AluOpType.mult,
                    )
                    mx = sp.tile([128, 1], F32, tag=f"mx{gi}")
                    nc.vector.tensor_reduce(
                        mx, act_t[:, ds(off, cw)], axis=mybir.AxisListType.X,
                        op=mybir.AluOpType.max, apply_absolute_value=True,
                    )
                    mxs.append(mx)
                m2 = sp.tile([128, 1], F32, tag="m2")
                nc.vector.tensor_tensor(m2, mxs[0], mxs[1], mybir.AluOpType.max)
                nc.vector.tensor_tensor(m2, m2, mxs[2], mybir.AluOpType.max)
                s_tr = sp.tile([128, 1], F32, tag="s_tr")
                nc.vector.tensor_scalar(
                    s_tr, m2, 1.0 / 127.0, 1e-8,
                    mybir.AluOpType.mult, mybir.AluOpType.max,
                )
                sc_eff = sp.tile([128, 1], F32, tag="sc_eff")  # 1 / s_a
                nc.vector.reciprocal(sc_eff, s_tr)
                c_t = sp.tile([128, 1], F32, tag="c")  # final row scale r * s_a
                nc.vector.tensor_tensor(
                    c_t, s_tr, r_sb[:, t : t + 1], mybir.AluOpType.mult
                )
                return act_t, sc_eff, c_t

            def requant(t, act_t, sc_eff):
                """Act: round(act' * sc_eff) -> bf16 ints; DMA-xbar transpose."""
                act_q = aqp.tile([128, I], BF16, tag="aq")
                for off, cw in RQ_CHUNKS:
                    t1 = rqp.tile([128, 512], F32, tag="rq")
                    nc.scalar.activation(
                        t1[:, :cw], act_t[:, ds(off, cw)],
                        mybir.ActivationFunctionType.Identity,
                        bias=pos223, scale=sc_eff,
                    )
                    nc.scalar.activation(
                        act_q[:, ds(off, cw)], t1[:, :cw],
                        mybir.ActivationFunctionType.Identity, bias=neg223,
                    )
                qa = qatp.tile([128, KT2, 128], BF16, tag="qa")
                nc.sync.dma_start_transpose(qa[:], act_q[:])
                return qa

            def fc2_half(t, qa, op_):
                """PE: one 1024-column half of the second int8 GEMM."""
                pa = ps2.tile([128, 512], F32, tag="psa")
                pb = ps2.tile([128, 512], F32, tag="psb")
                for k in range(KT2):
                    nc.tensor.matmul(
                        pa, qa[:, k, :], w2_sb[:, k, ts(2 * op_, 512)],
                        start=(k == 0), stop=(k == KT2 - 1),
                    )
                    nc.tensor.matmul(
                        pb, qa[:, k, :], w2_sb[:, k, ts(2 * op_ + 1, 512)],
                        start=(k == 0), stop=(k == KT2 - 1),
                    )
                return [(2 * op_, pa), (2 * op_ + 1, pb)]

            def fc2(t, qa):
                """PE: second int8 GEMM."""
                return fc2_half(t, qa, 0) + fc2_half(t, qa, 1)

            def store(t, ppairs, c_t):
                """DVE: dequant fc2 + per-token gate scale; DMA out."""
                t0 = t * 128
                for oc_i, pp in ppairs:
                    oc = oc_i * 512
                    yc = yp.tile([128, 512], F32, tag="yc")
                    nc.vector.tensor_tensor(
                        yc, pp, s2w_b[:, ds(oc, 512)], mybir.AluOpType.mult
                    )
                    nc.vector.tensor_scalar_mul(yc, yc, c_t[:, :])
                    nc.sync.dma_start(x_like(y_d)[t0 : t0 + 128, ds(oc, 512)], yc)

            # ---- software-pipelined tile loop ----
            state = {}
            if nt >= 3:
                # Prologue: braided fc1(0)+fc1(1).g0 (k-outer, all 8 PSUM
                # banks) rides the w13 DMA stream; fc1(2) is front-loaded so
                # the PE stays busy while the w2 stream lands before fc2(0).
                t0g, g0pair, t1_lag = fc1_braid()
                pgus1 = fc1_rest1(g0pair, t1_lag)
                state[0] = chain(0, t0g)
                state[1] = chain(1, pgus1)
                pgus2 = fc1(2)
                act_t, sc_eff, c_t = state.pop(0)
                qa = requant(0, act_t, sc_eff)
                ppairs = fc2(0, qa)
                state[2] = chain(2, pgus2)
                store(0, ppairs, c_t)
                if 3 < nt:
                    prep(3)
                for t in range(1, nt):
                    if t + 2 < nt:
                        pgus_n = fc1(t + 2)
                    act_t, sc_eff, c_t = state.pop(t)
                    qa = requant(t, act_t, sc_eff)
                    if t == nt - 1:
                        # final tile: interleave stores with fc2 so the
                        # kernel-tail drain starts as soon as possible
                        for op_ in range(2):
                            ppairs = fc2_half(t, qa, op_)
                            store(t, ppairs, c_t)
                        continue
                    ppairs = fc2(t, qa)
                    if t + 2 < nt:
                        state[t + 2] = chain(t + 2, pgus_n)
                    store(t, ppairs, c_t)
                    if t + 3 < nt:
                        prep(t + 3)
            else:
                pgus = fc1(0)
                state[0] = chain(0, pgus)
                for t in range(nt):
                    if t + 1 < nt:
                        pgus_n = fc1(t + 1)
                    act_t, sc_eff, c_t = state.pop(t)
                    qa = requant(t, act_t, sc_eff)
                    ppairs = fc2(t, qa)
                    if t + 1 < nt:
                        state[t + 1] = chain(t + 1, pgus_n)
                    store(t, ppairs, c_t)
                    if t + 2 < nt:
                        prep(t + 2)

    nc.finalize()
    return nc


def x_like(handle):
    """DRamTensorHandle -> AP covering the whole tensor."""
    return handle[:]


def _bcast128(handle, off, n):
    """AP reading handle[off:off+n] replicated across 128 partitions."""
    ap = handle[:][ds(off, n)]
    return bass.AP(tensor=ap.tensor, offset=ap.offset, ap=[[0, 128]] + list(ap.ap))


def _host_prep(hidden_states, gate_weight, w13_weight, w13_weight_scale,
               w2_weight, w2_weight_scale):
    """Host routing + quantization + per-core input maps. -> (in_maps, idxs, C)."""
    x = np.ascontiguousarray(np.asarray(hidden_states, dtype=np.float32))
    gw = np.asarray(gate_weight, dtype=np.float32)
    w13 = np.asarray(w13_weight)
    s13 = np.ascontiguousarray(np.asarray(w13_weight_scale, dtype=np.float32))
    w2 = np.asarray(w2_weight)
    s2w = np.ascontiguousarray(np.asarray(w2_weight_scale, dtype=np.float32))

    # ---- host routing: fp32 gate, softmax, top-2, renormalize ----
    logits = (x @ gw.T).astype(np.float32)
    p = np.exp(logits - logits.max(axis=1, keepdims=True), dtype=np.float32)
    p = (p / p.sum(axis=1, keepdims=True)).astype(np.float32)
    topi = np.argsort(-p, axis=1, kind="stable")[:, :TOPK]  # ties -> lower index
    topv = np.take_along_axis(p, topi, axis=1).astype(np.float32)
    gates = (topv / topv.sum(axis=1, keepdims=True)).astype(np.float32)

    # ---- host dynamic per-token int8 quant (exact reference replica) ----
    s_in = np.maximum(np.abs(x).max(axis=1, keepdims=True) / 127.0, 1e-8)
    s_in = s_in.astype(np.float32)
    q = np.clip(np.round(x / s_in), -127.0, 127.0).astype(np.float32)

    idxs, rvals = [], []
    for e in range(E):
        sel = topi == e
        tok = np.nonzero(sel.any(axis=1))[0]
        r = (gates * sel)[tok].sum(axis=1).astype(np.float32)
        idxs.append(tok)
        rvals.append(r)

    cap = max(128, max(len(t) for t in idxs))
    C = ((cap + 127) // 128) * 128
    nt = C // 128

    q_bf = q.astype(ml_dtypes.bfloat16)
    in_maps = []
    for e in range(E):
        n_e = len(idxs[e])
        qg = np.zeros((C, H), dtype=ml_dtypes.bfloat16)
        qg[:n_e] = q_bf[idxs[e]]
        # qt[t, p, k, j] = qg[t*128 + j, 128*k + p]
        qt = np.ascontiguousarray(
            qg.T.reshape(KT1, 128, nt, 128).transpose(2, 1, 0, 3)
        )
        sin_flat = np.zeros(C, dtype=np.float32)
        sin_flat[:n_e] = s_in[idxs[e], 0]
        sin_pack = np.ascontiguousarray(sin_flat.reshape(nt, 128).T)
        r_flat = np.zeros(C, dtype=np.float32)
        r_flat[:n_e] = rvals[e]
        r_pack = np.ascontiguousarray(r_flat.reshape(nt, 128).T)
        w13t = np.ascontiguousarray(w13[e].T).reshape(KT1, 128, 2 * I)
        w2t = np.ascontiguousarray(w2[e].T).reshape(KT2, 128, H)
        in_maps.append({
            "qt": qt,
            "sin": sin_pack,
            "r": r_pack,
            "w13t": w13t,
            "w2t": w2t,
            "s13": np.ascontiguousarray(s13[e]),
            "s2w": np.ascontiguousarray(s2w[e]),
        })
    return in_maps, idxs, C


def kernel(hidden_states, gate_weight, w13_weight, w13_weight_scale,
           w2_weight, w2_weight_scale):
    in_maps, idxs, C = _host_prep(hidden_states, gate_weight, w13_weight,
                                  w13_weight_scale, w2_weight, w2_weight_scale)
    if C not in _cache:
        _cache[C] = _build_program(C)
    nc = _cache[C]

    trace = bool(int(os.environ.get("MOE_TRACE", "0")))
    br = run_bass_kernel_spmd(nc, in_maps, list(range(E)), trace=trace)
    global LAST_EXEC_NS
    LAST_EXEC_NS = br.exec_time_ns
    res = br.results

    out = np.zeros((T, H), dtype=np.float32)
    for e in range(E):
        n_e = len(idxs[e])
        if n_e:
            out[idxs[e]] += np.asarray(res[e]["y"])[:n_e]
    return out


# revision 39
# speedup vs baseline: 1.0068x; 1.0004x over previous
"""Trainium2 Bass kernel for nn_IxformerQuantMoe (quantized top-2 MoE, E=8 experts).

Strategy (expert-parallel across 8 NeuronCores):
  - Host computes the fp32 gate (softmax + top-2 + renormalize), the per-token
    dynamic int8 quantization of hidden_states (exact numpy replica of the
    reference), and routes tokens: for each expert e, gathers the quantized
    rows of the tokens whose top-2 contains e, padded to a common capacity C
    (multiple of 128), pre-transposed to the contraction-major tile layout
    the PE array consumes.
  - Core e runs the expert FFN for its token set:
      int8 GEMM fc1 (exact in bf16, fp32 PSUM) -> dequant+SwiGLU -> dynamic
      requant (round-to-nearest via the 1.5*2^23 trick) -> DMA-xbar transpose
      -> int8 GEMM fc2 -> dequant + gate scaling.
  - Throughput structure (CoreSim: PE ~95% busy):
      * per-tile work is software-pipelined one tile ahead (fc1 of tile t+1
        runs while tile t requants/fc2s) so the PE never waits on the
        dequant/requant chain;
      * weights ship as int8 (half the HBM bytes) and are up-converted to
        bf16 on-chip on the otherwise-idle DVE/GpSimd engines;
      * the prologue runs fc1(0) k-OUTER braided with fc1(1).group0 across
        all 8 PSUM banks so the PE consumes w13 k-tiles in DMA arrival
        order, and fc1(2) is front-loaded before fc2(0) to cover the w2
        stream;
      * the dequant applies (psum * s_in) first (reference association),
        which also frees PSUM banks without waiting on the s13 scale DMAs.
  - Host scatter-adds each expert's output rows into the final [T, H] output
    (the weighted top-2 combine).
"""

import os
import sys

for _p in ("/opt/trn_rl_repo", "/root/.axon_site/_ro/trn_rl_repo"):
    if os.path.isdir(_p) and _p not in sys.path:
        sys.path.insert(0, _p)

import numpy as np
import ml_dtypes

import concourse.bass as bass
import concourse.bacc as bacc
import concourse.tile as tile
from concourse import mybir
from concourse.bass import ds, ts
from concourse.bass_utils import run_bass_kernel_spmd

T, H, I, E, TOPK = 4096, 2048, 1408, 8, 2
KT1 = H // 128     # 16 k-tiles for fc1 contraction
KT2 = I // 128     # 11 k-tiles for fc2 contraction
TWO23 = 12582912.0  # 1.5*2^23: fp32 add/sub rounds to nearest integer (RNE)

F32 = mybir.dt.float32
BF16 = mybir.dt.bfloat16
I8 = mybir.dt.int8

FC1_GROUPS = [(0, 512), (512, 512), (1024, 384)]   # column groups over I
RQ_CHUNKS = [(0, 512), (512, 512), (1024, 384)]    # requant chunks over I

_cache = {}
LAST_EXEC_NS = None


def _build_program(C):
    """Bass program run identically (SPMD) on 8 cores; per-core data differs."""
    nt = C // 128
    nc = bacc.Bacc(None, target_bir_lowering=False)

    qt_d = nc.declare_dram_parameter("qt", [nt, 128, KT1, 128], BF16, isOutput=False)
    sin_d = nc.declare_dram_parameter("sin", [128, nt], F32, isOutput=False)
    r_d = nc.declare_dram_parameter("r", [128, nt], F32, isOutput=False)
    w13_d = nc.declare_dram_parameter("w13t", [KT1, 128, 2 * I], I8, isOutput=False)
    w2_d = nc.declare_dram_parameter("w2t", [KT2, 128, H], I8, isOutput=False)
    s13_d = nc.declare_dram_parameter("s13", [2 * I], F32, isOutput=False)
    s2w_d = nc.declare_dram_parameter("s2w", [H], F32, isOutput=False)
    y_d = nc.declare_dram_parameter("y", [C, H], F32, isOutput=True)

    with tile.TileContext(nc) as tc:
        with (
            tc.tile_pool(name="singles", bufs=1) as singles,
            tc.tile_pool(name="qtp", bufs=2) as qtp,
            tc.tile_pool(name="gp", bufs=2) as gp,
            tc.tile_pool(name="actp", bufs=2) as actp,
            tc.tile_pool(name="aqp", bufs=2) as aqp,
            tc.tile_pool(name="rqp", bufs=2) as rqp,
            tc.tile_pool(name="qatp", bufs=2) as qatp,
            tc.tile_pool(name="yp", bufs=2) as yp,
            tc.tile_pool(name="wsp", bufs=2) as wsp,
            tc.tile_pool(name="sp", bufs=3) as sp,
            tc.tile_pool(name="ps1", bufs=2, space="PSUM") as ps1,
            tc.tile_pool(name="ps2", bufs=2, space="PSUM") as ps2,
        ):
            # ---- resident tiles ----
            w13_sb = singles.tile([128, KT1, 2 * I], BF16)
            w2_sb = singles.tile([128, KT2, H], BF16)
            s13g_b = singles.tile([128, I], F32)
            s13u_b = singles.tile([128, I], F32)
            s2w_b = singles.tile([128, H], F32)
            sin_sb = singles.tile([128, nt], F32)
            r_sb = singles.tile([128, nt], F32)

            neg223 = singles.tile([128, 1], F32)
            nc.vector.memset(neg223, -TWO23)
            pos223 = singles.tile([128, 1], F32)
            nc.vector.memset(pos223, TWO23)
            zero_b = singles.tile([128, 1], F32)
            nc.vector.memset(zero_b, 0.0)

            qts = {}

            def prep(t, split=False):
                """DMA the pre-transposed quantized input tile for token-tile t."""
                qt = qtp.tile([128, KT1, 128], BF16, tag="qt")
                if split:
                    # two halves so the braid's k=0 matmuls start sooner
                    half = KT1 // 2
                    nc.sync.dma_start(qt[:, :half, :], x_like(qt_d)[t][:, :half, :])
                    nc.sync.dma_start(qt[:, half:, :], x_like(qt_d)[t][:, half:, :])
                else:
                    nc.sync.dma_start(qt, x_like(qt_d)[t])
                qts[t] = qt

            # ---- prologue DMA order: int8 weights streamed and up-converted
            # on-chip (w13 alternating DVE/Act, w2 on GpSimd) so fc1(0) can
            # start as soon as w13 k-tile 0 lands.
            # warm the PE p-state/HAM during the initial DMA wait with dummy
            # matmuls on (junk) SBUF; the result is never read
            wmup = ps2.tile([128, 512], F32, tag="psa")
            for _ in range(9):
                nc.tensor.matmul(
                    wmup[:1, :], w13_sb[:, 0, 0:1], w13_sb[:, 1, 0:512],
                    start=True, stop=True,
                )

            # k0 specially split (and interleaved with qt(0)'s halves) so the
            # braid's first g-side matmuls start as early as possible
            qt0 = qtp.tile([128, KT1, 128], BF16, tag="qt")
            qts[0] = qt0
            nc.sync.dma_start(qt0[:, : KT1 // 2, :], x_like(qt_d)[0][:, : KT1 // 2, :])
            ws0 = wsp.tile([128, 2 * I], I8, tag="wse")
            nc.sync.dma_start(ws0[:, :I], x_like(w13_d)[0][:, :I])
            nc.vector.tensor_copy(w13_sb[:, 0, :I], ws0[:, :I])
            nc.sync.dma_start(ws0[:, I:], x_like(w13_d)[0][:, I:])
            nc.vector.tensor_copy(w13_sb[:, 0, I:], ws0[:, I:])
            nc.sync.dma_start(qt0[:, KT1 // 2 :, :], x_like(qt_d)[0][:, KT1 // 2 :, :])
            for k in range(1, KT1):
                ws = wsp.tile([128, 2 * I], I8, tag=("wse" if k % 2 == 0 else "wso"))
                nc.sync.dma_start(ws, x_like(w13_d)[k])
                # the braid-critical early k-tiles convert on the faster DVE
                # (g/u halves, since the braid's pg matmuls only need g);
                # odd tiles from k5 go to GpSimd, which has more slack
                if k <= 3:
                    nc.vector.tensor_copy(w13_sb[:, k, :I], ws[:, :I])
                    nc.vector.tensor_copy(w13_sb[:, k, I:], ws[:, I:])
                elif k % 2 == 0:
                    nc.vector.tensor_copy(w13_sb[:, k, :], ws)
                else:
                    nc.gpsimd.tensor_copy(w13_sb[:, k, :], ws)
                if k == 5 and nt > 1:
                    prep(1)
            nc.sync.dma_start(sin_sb, x_like(sin_d))
            nc.sync.dma_start(r_sb, x_like(r_d))
            nc.sync.dma_start(s13g_b, _bcast128(s13_d, 0, I))
            nc.sync.dma_start(s13u_b, _bcast128(s13_d, I, I))
            if nt > 2:
                prep(2)
            for k in range(KT2):
                ws = wsp.tile([128, 2 * I], I8, tag=("wse" if k % 2 == 0 else "wso"))
                nc.sync.dma_start(ws[:, :H], x_like(w2_d)[k])
                nc.gpsimd.tensor_copy(w2_sb[:, k, :], ws[:, :H])
            nc.sync.dma_start(s2w_b, _bcast128(s2w_d, 0, H))

            def fc1(t):
                """PE: int8-as-bf16 GEMM into PSUM group pairs."""
                pgus = []
                for off, cw in FC1_GROUPS:
                    pg = ps1.tile([128, 512], F32, tag="psg")
                    pu = ps1.tile([128, 512], F32, tag="psu")
                    for k in range(KT1):
                        nc.tensor.matmul(
                            pg[:, :cw], qts[t][:, k, :], w13_sb[:, k, ds(off, cw)],
                            start=(k == 0), stop=(k == KT1 - 1),
                        )
                        nc.tensor.matmul(
                            pu[:, :cw], qts[t][:, k, :], w13_sb[:, k, ds(I + off, cw)],
                            start=(k == 0), stop=(k == KT1 - 1),
                        )
                    pgus.append((off, cw, pg, pu))
                del qts[t]
                return pgus

            def fc1_braid():
                """Prologue fc1 for tiles 0 and 1-group0, k-outer so the PE
                consumes w13 k-tiles in DMA arrival order (all 8 PSUM banks)."""
                t0g = []
                for gi, (off, cw) in enumerate(FC1_GROUPS):
                    pg = (ps1 if gi < 2 else ps2).tile(
                        [128, 512], F32, tag=("psg" if gi < 2 else "psa"))
                    pu = (ps1 if gi < 2 else ps2).tile(
                        [128, 512], F32, tag=("psu" if gi < 2 else "psb"))
                    t0g.append((off, cw, pg, pu))
                pgB = ps2.tile([128, 512], F32, tag="psa")
                puB = ps2.tile([128, 512], F32, tag="psb")
                # tile 1's matmuls join the braid 4 steps late (its qt DMA
                # lands after w13 k-tile 5); the last k-tiles finish in rest1
                T1_LAG = 4
                for ki in range(KT1):
                    k = ki
                    for off, cw, pg, pu in t0g:
                        nc.tensor.matmul(
                            pg[:, :cw], qts[0][:, k, :], w13_sb[:, k, ds(off, cw)],
                            start=(k == 0), stop=(k == KT1 - 1),
                        )
                        nc.tensor.matmul(
                            pu[:, :cw], qts[0][:, k, :], w13_sb[:, k, ds(I + off, cw)],
                            start=(k == 0), stop=(k == KT1 - 1),
                        )
                    if ki < T1_LAG:
                        continue
                    kb = ki - T1_LAG
                    off, cw = FC1_GROUPS[0]
                    nc.tensor.matmul(
                        pgB[:, :cw], qts[1][:, kb, :], w13_sb[:, kb, ds(off, cw)],
                        start=(kb == 0), stop=False,
                    )
                    nc.tensor.matmul(
                        puB[:, :cw], qts[1][:, kb, :], w13_sb[:, kb, ds(I + off, cw)],
                        start=(kb == 0), stop=False,
                    )
                del qts[0]
                return t0g, (FC1_GROUPS[0][0], FC1_GROUPS[0][1], pgB, puB), T1_LAG

            def fc1_rest1(g0pair, t1_lag):
                """Tile 1: finish group 0's lagged k-tiles, then groups 1..2."""
                off, cw, pgB, puB = g0pair
                for kb in range(KT1 - t1_lag, KT1):
                    nc.tensor.matmul(
                        pgB[:, :cw], qts[1][:, kb, :], w13_sb[:, kb, ds(off, cw)],
                        start=False, stop=(kb == KT1 - 1),
                    )
                    nc.tensor.matmul(
                        puB[:, :cw], qts[1][:, kb, :], w13_sb[:, kb, ds(I + off, cw)],
                        start=False, stop=(kb == KT1 - 1),
                    )
                pgus = [g0pair]
                for off, cw in FC1_GROUPS[1:]:
                    pg = ps1.tile([128, 512], F32, tag="psg")
                    pu = ps1.tile([128, 512], F32, tag="psu")
                    for k in range(KT1):
                        nc.tensor.matmul(
                            pg[:, :cw], qts[1][:, k, :], w13_sb[:, k, ds(off, cw)],
                            start=(k == 0), stop=(k == KT1 - 1),
                        )
                        nc.tensor.matmul(
                            pu[:, :cw], qts[1][:, k, :], w13_sb[:, k, ds(I + off, cw)],
                            start=(k == 0), stop=(k == KT1 - 1),
                        )
                    pgus.append((off, cw, pg, pu))
                del qts[1]
                return pgus

            def chain(t, pgus):
                """DVE/Act: dequant + SwiGLU + abs-max + requant scales.

                g = (pg * s_in) * s13g ; u = (pu * s_in) * s13u  (reference
                association); act = silu(g) * u; s_a = max|act|/127.
                The s_in tensor_scalar comes first so the PSUM bank frees
                without waiting on the s13 broadcast DMAs.
                """
                sin_col = sin_sb[:, t : t + 1]
                act_t = actp.tile([128, I], F32, tag="act")
                mxs = []
                for gi, (off, cw, pg, pu) in enumerate(pgus):
                    g_sc = gp.tile([128, 512], F32, tag="gsc")
                    nc.vector.tensor_scalar_mul(g_sc[:, :cw], pg[:, :cw], sin_col)
                    nc.vector.tensor_tensor(
                        g_sc[:, :cw], g_sc[:, :cw], s13g_b[:, ds(off, cw)],
                        mybir.AluOpType.mult,
                    )
                    nc.scalar.activation(
                        g_sc[:, :cw], g_sc[:, :cw],
                        mybir.ActivationFunctionType.Silu, bias=zero_b, scale=1.0,
                    )
                    nc.vector.tensor_scalar_mul(
                        act_t[:, ds(off, cw)], pu[:, :cw], sin_col
                    )
                    nc.vector.tensor_tensor(
                        act_t[:, ds(off, cw)], act_t[:, ds(off, cw)],
                        s13u_b[:, ds(off, cw)], mybir.AluOpType.mult,
                    )
                    nc.vector.tensor_tensor(
                        act_t[:, ds(off, cw)], act_t[:, ds(off, cw)], g_sc[:, :cw],
                        mybir.AluOpType.mult,
                    )
                    mx = sp.tile([128, 1], F32, tag=f"mx{gi}")
                    nc.vector.tensor_reduce(
                        mx, act_t[:, ds(off, cw)], axis=mybir.AxisListType.X,
                        op=mybir.AluOpType.max, apply_absolute_value=True,
                    )
                    mxs.append(mx)
                m2 = sp.tile([128, 1], F32, tag="m2")
                nc.vector.tensor_tensor(m2, mxs[0], mxs[1], mybir.AluOpType.max)
                nc.vector.tensor_tensor(m2, m2, mxs[2], mybir.AluOpType.max)
                s_tr = sp.tile([128, 1], F32, tag="s_tr")
                nc.vector.tensor_scalar(
                    s_tr, m2, 1.0 / 127.0, 1e-8,
                    mybir.AluOpType.mult, mybir.AluOpType.max,
                )
                sc_eff = sp.tile([128, 1], F32, tag="sc_eff")  # 1 / s_a
                nc.vector.reciprocal(sc_eff, s_tr)
                c_t = sp.tile([128, 1], F32, tag="c")  # final row scale r * s_a
                nc.vector.tensor_tensor(
                    c_t, s_tr, r_sb[:, t : t + 1], mybir.AluOpType.mult
                )
                return act_t, sc_eff, c_t

            def requant(t, act_t, sc_eff):
                """Act: round(act' * sc_eff) -> bf16 ints; DMA-xbar transpose."""
                act_q = aqp.tile([128, I], BF16, tag="aq")
                for off, cw in RQ_CHUNKS:
                    t1 = rqp.tile([128, 512], F32, tag="rq")
                    nc.scalar.activation(
                        t1[:, :cw], act_t[:, ds(off, cw)],
                        mybir.ActivationFunctionType.Identity,
                        bias=pos223, scale=sc_eff,
                    )
                    nc.scalar.activation(
                        act_q[:, ds(off, cw)], t1[:, :cw],
                        mybir.ActivationFunctionType.Identity, bias=neg223,
                    )
                qa = qatp.tile([128, KT2, 128], BF16, tag="qa")
                nc.sync.dma_start_transpose(qa[:], act_q[:])
                return qa

            def fc2_half(t, qa, op_):
                """PE: one 1024-column half of the second int8 GEMM."""
                pa = ps2.tile([128, 512], F32, tag="psa")
                pb = ps2.tile([128, 512], F32, tag="psb")
                for k in range(KT2):
                    nc.tensor.matmul(
                        pa, qa[:, k, :], w2_sb[:, k, ts(2 * op_, 512)],
                        start=(k == 0), stop=(k == KT2 - 1),
                    )
                    nc.tensor.matmul(
                        pb, qa[:, k, :], w2_sb[:, k, ts(2 * op_ + 1, 512)],
                        start=(k == 0), stop=(k == KT2 - 1),
                    )
                return [(2 * op_, pa), (2 * op_ + 1, pb)]

            def fc2(t, qa):
                """PE: second int8 GEMM."""
                return fc2_half(t, qa, 0) + fc2_half(t, qa, 1)

            def store(t, ppairs, c_t):
                """DVE: dequant fc2 + per-token gate scale; DMA out."""
                t0 = t * 128
                for oc_i, pp in ppairs:
                    oc = oc_i * 512
                    yc = yp.tile([128, 512], F32, tag="yc")
                    nc.vector.tensor_tensor(
                        yc, pp, s2w_b[:, ds(oc, 512)], mybir.AluOpType.mult
                    )
                    nc.vector.tensor_scalar_mul(yc, yc, c_t[:, :])
                    nc.sync.dma_start(x_like(y_d)[t0 : t0 + 128, ds(oc, 512)], yc)

            # ---- software-pipelined tile loop ----
            state = {}
            if nt >= 3:
                # Prologue: braided fc1(0)+fc1(1).g0 (k-outer, all 8 PSUM
                # banks) rides the w13 DMA stream; fc1(2) is front-loaded so
                # the PE stays busy while the w2 stream lands before fc2(0).
                t0g, g0pair, t1_lag = fc1_braid()
                pgus1 = fc1_rest1(g0pair, t1_lag)
                state[0] = chain(0, t0g)
                state[1] = chain(1, pgus1)
                pgus2 = fc1(2)
                act_t, sc_eff, c_t = state.pop(0)
                qa = requant(0, act_t, sc_eff)
                ppairs = fc2(0, qa)
                state[2] = chain(2, pgus2)
                store(0, ppairs, c_t)
                if 3 < nt:
                    prep(3)
                for t in range(1, nt):
                    if t + 2 < nt:
                        pgus_n = fc1(t + 2)
                    act_t, sc_eff, c_t = state.pop(t)
                    qa = requant(t, act_t, sc_eff)
                    if t == nt - 1:
                        # final tile: interleave stores with fc2 so the
                        # kernel-tail drain starts as soon as possible
                        for op_ in range(2):
                            ppairs = fc2_half(t, qa, op_)
                            store(t, ppairs, c_t)
                        continue
                    ppairs = fc2(t, qa)
                    if t + 2 < nt:
                        state[t + 2] = chain(t + 2, pgus_n)
                    store(t, ppairs, c_t)
                    if t + 3 < nt:
                        prep(t + 3)
            else:
                pgus = fc1(0)
                state[0] = chain(0, pgus)
                for t in range(nt):
                    if t + 1 < nt:
                        pgus_n = fc1(t + 1)
                    act_t, sc_eff, c_t = state.pop(t)
                    qa = requant(t, act_t, sc_eff)
                    ppairs = fc2(t, qa)
                    if t + 1 < nt:
                        state[t + 1] = chain(t + 1, pgus_n)
                    store(t, ppairs, c_t)
                    if t + 2 < nt:
                        prep(t + 2)

    nc.finalize()
    return nc


def x_like(handle):
    """DRamTensorHandle -> AP covering the whole tensor."""
    return handle[:]


def _bcast128(handle, off, n):
    """AP reading handle[off:off+n] replicated across 128 partitions."""
    ap = handle[:][ds(off, n)]
    return bass.AP(tensor=ap.tensor, offset=ap.offset, ap=[[0, 128]] + list(ap.ap))


def _host_prep(hidden_states, gate_weight, w13_weight, w13_weight_scale,
               w2_weight, w2_weight_scale):
    """Host routing + quantization + per-core input maps. -> (in_maps, idxs, C)."""
    x = np.ascontiguousarray(np.asarray(hidden_states, dtype=np.float32))
    gw = np.asarray(gate_weight, dtype=np.float32)
    w13 = np.asarray(w13_weight)
    s13 = np.ascontiguousarray(np.asarray(w13_weight_scale, dtype=np.float32))
    w2 = np.asarray(w2_weight)
    s2w = np.ascontiguousarray(np.asarray(w2_weight_scale, dtype=np.float32))

    # ---- host routing: fp32 gate, softmax, top-2, renormalize ----
    logits = (x @ gw.T).astype(np.float32)
    p = np.exp(logits - logits.max(axis=1, keepdims=True), dtype=np.float32)
    p = (p / p.sum(axis=1, keepdims=True)).astype(np.float32)
    topi = np.argsort(-p, axis=1, kind="stable")[:, :TOPK]  # ties -> lower index
    topv = np.take_along_axis(p, topi, axis=1).astype(np.float32)
    gates = (topv / topv.sum(axis=1, keepdims=True)).astype(np.float32)

    # ---- host dynamic per-token int8 quant (exact reference replica) ----
    s_in = np.maximum(np.abs(x).max(axis=1, keepdims=True) / 127.0, 1e-8)
    s_in = s_in.astype(np.float32)
    q = np.clip(np.round(x / s_in), -127.0, 127.0).astype(np.float32)

    idxs, rvals = [], []
    for e in range(E):
        sel = topi == e
        tok = np.nonzero(sel.any(axis=1))[0]
        r = (gates * sel)[tok].sum(axis=1).astype(np.float32)
        idxs.append(tok)
        rvals.append(r)

    cap = max(128, max(len(t) for t in idxs))
    C = ((cap + 127) // 128) * 128
    nt = C // 128

    q_bf = q.astype(ml_dtypes.bfloat16)
    in_maps = []
    for e in range(E):
        n_e = len(idxs[e])
        qg = np.zeros((C, H), dtype=ml_dtypes.bfloat16)
        qg[:n_e] = q_bf[idxs[e]]
        # qt[t, p, k, j] = qg[t*128 + j, 128*k + p]
        qt = np.ascontiguousarray(
            qg.T.reshape(KT1, 128, nt, 128).transpose(2, 1, 0, 3)
        )
        sin_flat = np.zeros(C, dtype=np.float32)
        sin_flat[:n_e] = s_in[idxs[e], 0]
        sin_pack = np.ascontiguousarray(sin_flat.reshape(nt, 128).T)
        r_flat = np.zeros(C, dtype=np.float32)
        r_flat[:n_e] = rvals[e]
        r_pack = np.ascontiguousarray(r_flat.reshape(nt, 128).T)
        w13t = np.ascontiguousarray(w13[e].T).reshape(KT1, 128, 2 * I)
        w2t = np.ascontiguousarray(w2[e].T).reshape(KT2, 128, H)
        in_maps.append({
            "qt": qt,
            "sin": sin_pack,
            "r": r_pack,
            "w13t": w13t,
            "w2t": w2t,
            "s13": np.ascontiguousarray(s13[e]),
            "s2w": np.ascontiguousarray(s2w[e]),
        })
    return in_maps, idxs, C


def kernel(hidden_states, gate_weight, w13_weight, w13_weight_scale,
           w2_weight, w2_weight_scale):
    in_maps, idxs, C = _host_prep(hidden_states, gate_weight, w13_weight,
                                  w13_weight_scale, w2_weight, w2_weight_scale)
    if C not in _cache:
        _cache[C] = _build_program(C)
    nc = _cache[C]

    trace = bool(int(os.environ.get("MOE_TRACE", "0")))
    br = run_bass_kernel_spmd(nc, in_maps, list(range(E)), trace=trace)
    global LAST_EXEC_NS
    LAST_EXEC_NS = br.exec_time_ns
    res = br.results

    out = np.zeros((T, H), dtype=np.float32)
    for e in range(E):
        n_e = len(idxs[e])
        if n_e:
            out[idxs[e]] += np.asarray(res[e]["y"])[:n_e]
    return out


# revision 57
# speedup vs baseline: 1.0086x; 1.0017x over previous
"""Trainium2 Bass kernel for nn_IxformerQuantMoe (quantized top-2 MoE, E=8 experts).

Strategy (expert-parallel across 8 NeuronCores):
  - Host computes the fp32 gate (softmax + top-2 + renormalize), the per-token
    dynamic int8 quantization of hidden_states (exact numpy replica of the
    reference), and routes tokens: for each expert e, gathers the quantized
    rows of the tokens whose top-2 contains e, padded to a common capacity C
    (multiple of 128), pre-transposed to the contraction-major tile layout
    the PE array consumes.
  - Core e runs the expert FFN for its token set:
      int8 GEMM fc1 (exact in bf16, fp32 PSUM) -> dequant+SwiGLU -> dynamic
      requant (round-to-nearest via the 1.5*2^23 trick) -> DMA-xbar transpose
      -> int8 GEMM fc2 -> dequant + gate scaling.
  - Throughput structure (CoreSim: PE ~95% busy):
      * per-tile work is software-pipelined one tile ahead (fc1 of tile t+1
        runs while tile t requants/fc2s) so the PE never waits on the
        dequant/requant chain;
      * weights ship as int8 (half the HBM bytes) and are up-converted to
        bf16 on-chip on the otherwise-idle DVE/GpSimd engines;
      * the prologue runs fc1(0) k-OUTER braided with fc1(1).group0 across
        all 8 PSUM banks so the PE consumes w13 k-tiles in DMA arrival
        order, and fc1(2) is front-loaded before fc2(0) to cover the w2
        stream;
      * the dequant applies (psum * s_in) first (reference association),
        which also frees PSUM banks without waiting on the s13 scale DMAs.
  - Host scatter-adds each expert's output rows into the final [T, H] output
    (the weighted top-2 combine).
"""

import os
import sys

for _p in ("/opt/trn_rl_repo", "/root/.axon_site/_ro/trn_rl_repo"):
    if os.path.isdir(_p) and _p not in sys.path:
        sys.path.insert(0, _p)

import numpy as np
import ml_dtypes

import concourse.bass as bass
import concourse.bacc as bacc
import concourse.tile as tile
from concourse import mybir
from concourse.bass import ds, ts
from concourse.bass_utils import run_bass_kernel_spmd

T, H, I, E, TOPK = 4096, 2048, 1408, 8, 2
KT1 = H // 128     # 16 k-tiles for fc1 contraction
KT2 = I // 128     # 11 k-tiles for fc2 contraction
TWO23 = 12582912.0  # 1.5*2^23: fp32 add/sub rounds to nearest integer (RNE)

F32 = mybir.dt.float32
BF16 = mybir.dt.bfloat16
I8 = mybir.dt.int8

FC1_GROUPS = [(0, 512), (512, 512), (1024, 384)]   # column groups over I
RQ_CHUNKS = [(0, 512), (512, 512), (1024, 384)]    # requant chunks over I

_cache = {}
LAST_EXEC_NS = None


def _build_program(C):
    """Bass program run identically (SPMD) on 8 cores; per-core data differs."""
    nt = C // 128
    nc = bacc.Bacc(None, target_bir_lowering=False)

    qt_d = nc.declare_dram_parameter("qt", [nt, 128, KT1, 128], BF16, isOutput=False)
    sin_d = nc.declare_dram_parameter("sin", [128, nt], F32, isOutput=False)
    r_d = nc.declare_dram_parameter("r", [128, nt], F32, isOutput=False)
    w13_d = nc.declare_dram_parameter("w13t", [KT1, 128, 2 * I], I8, isOutput=False)
    w2_d = nc.declare_dram_parameter("w2t", [KT2, 128, H], I8, isOutput=False)
    s13_d = nc.declare_dram_parameter("s13", [2 * I], F32, isOutput=False)
    s2w_d = nc.declare_dram_parameter("s2w", [H], F32, isOutput=False)
    y_d = nc.declare_dram_parameter("y", [C, H], F32, isOutput=True)

    with tile.TileContext(nc) as tc:
        with (
            tc.tile_pool(name="singles", bufs=1) as singles,
            tc.tile_pool(name="qtp", bufs=2) as qtp,
            tc.tile_pool(name="gp", bufs=2) as gp,
            tc.tile_pool(name="actp", bufs=2) as actp,
            tc.tile_pool(name="aqp", bufs=2) as aqp,
            tc.tile_pool(name="rqp", bufs=2) as rqp,
            tc.tile_pool(name="qatp", bufs=2) as qatp,
            tc.tile_pool(name="yp", bufs=2) as yp,
            tc.tile_pool(name="wsp", bufs=2) as wsp,
            tc.tile_pool(name="sp", bufs=3) as sp,
            tc.tile_pool(name="ps1", bufs=2, space="PSUM") as ps1,
            tc.tile_pool(name="ps2", bufs=2, space="PSUM") as ps2,
        ):
            # ---- resident tiles ----
            w13_sb = singles.tile([128, KT1, 2 * I], BF16)
            w2_sb = singles.tile([128, KT2, H], BF16)
            s13g_b = singles.tile([128, I], F32)
            s13u_b = singles.tile([128, I], F32)
            s2w_b = singles.tile([128, H], F32)
            sin_sb = singles.tile([128, nt], F32)
            r_sb = singles.tile([128, nt], F32)

            neg223 = singles.tile([128, 1], F32)
            nc.vector.memset(neg223, -TWO23)
            pos223 = singles.tile([128, 1], F32)
            nc.vector.memset(pos223, TWO23)
            zero_b = singles.tile([128, 1], F32)
            nc.vector.memset(zero_b, 0.0)

            qts = {}

            def prep(t, split=False):
                """DMA the pre-transposed quantized input tile for token-tile t."""
                qt = qtp.tile([128, KT1, 128], BF16, tag="qt")
                if split:
                    # two halves so the braid's k=0 matmuls start sooner
                    half = KT1 // 2
                    nc.sync.dma_start(qt[:, :half, :], x_like(qt_d)[t][:, :half, :])
                    nc.sync.dma_start(qt[:, half:, :], x_like(qt_d)[t][:, half:, :])
                else:
                    nc.sync.dma_start(qt, x_like(qt_d)[t])
                qts[t] = qt

            # ---- prologue DMA order: int8 weights streamed and up-converted
            # on-chip (w13 alternating DVE/Act, w2 on GpSimd) so fc1(0) can
            # start as soon as w13 k-tile 0 lands.
            # warm the PE p-state/HAM during the initial DMA wait with dummy
            # matmuls on (junk) SBUF; the result is never read
            wmup = ps2.tile([128, 512], F32, tag="psa")
            for _ in range(9):
                nc.tensor.matmul(
                    wmup[:1, :], w13_sb[:, 0, 0:1], w13_sb[:, 1, 0:512],
                    start=True, stop=True,
                )

            # k0 specially split (and interleaved with qt(0)'s halves) so the
            # braid's first g-side matmuls start as early as possible
            qt0 = qtp.tile([128, KT1, 128], BF16, tag="qt")
            qts[0] = qt0
            nc.sync.dma_start(qt0[:, : KT1 // 2, :], x_like(qt_d)[0][:, : KT1 // 2, :])
            ws0 = wsp.tile([128, 2 * I], I8, tag="wse")
            nc.sync.dma_start(ws0[:, :I], x_like(w13_d)[0][:, :I])
            nc.vector.tensor_copy(w13_sb[:, 0, :I], ws0[:, :I])
            nc.sync.dma_start(ws0[:, I:], x_like(w13_d)[0][:, I:])
            nc.vector.tensor_copy(w13_sb[:, 0, I:], ws0[:, I:])
            nc.sync.dma_start(qt0[:, KT1 // 2 :, :], x_like(qt_d)[0][:, KT1 // 2 :, :])
            for k in range(1, KT1):
                ws = wsp.tile([128, 2 * I], I8, tag=("wse" if k % 2 == 0 else "wso"))
                nc.sync.dma_start(ws, x_like(w13_d)[k])
                # the braid-critical early k-tiles convert on the faster DVE
                # (g/u halves, since the braid's pg matmuls only need g);
                # odd tiles from k5 go to GpSimd, which has more slack
                if k <= 3:
                    nc.vector.tensor_copy(w13_sb[:, k, :I], ws[:, :I])
                    nc.vector.tensor_copy(w13_sb[:, k, I:], ws[:, I:])
                elif k % 2 == 0:
                    nc.vector.tensor_copy(w13_sb[:, k, :], ws)
                else:
                    nc.gpsimd.tensor_copy(w13_sb[:, k, :], ws)
                if k == 5 and nt > 1:
                    prep(1)
            nc.sync.dma_start(sin_sb, x_like(sin_d))
            nc.sync.dma_start(r_sb, x_like(r_d))
            nc.sync.dma_start(s13g_b, _bcast128(s13_d, 0, I))
            nc.sync.dma_start(s13u_b, _bcast128(s13_d, I, I))
            if nt > 2:
                prep(2)
            for k in range(KT2):
                ws = wsp.tile([128, 2 * I], I8, tag=("wse" if k % 2 == 0 else "wso"))
                nc.sync.dma_start(ws[:, :H], x_like(w2_d)[k])
                nc.gpsimd.tensor_copy(w2_sb[:, k, :], ws[:, :H])
            nc.sync.dma_start(s2w_b, _bcast128(s2w_d, 0, H))

            def fc1(t):
                """PE: int8-as-bf16 GEMM into PSUM group pairs."""
                pgus = []
                for off, cw in FC1_GROUPS:
                    pg = ps1.tile([128, 512], F32, tag="psg")
                    pu = ps1.tile([128, 512], F32, tag="psu")
                    for k in range(KT1):
                        nc.tensor.matmul(
                            pg[:, :cw], qts[t][:, k, :], w13_sb[:, k, ds(off, cw)],
                            start=(k == 0), stop=(k == KT1 - 1),
                        )
                        nc.tensor.matmul(
                            pu[:, :cw], qts[t][:, k, :], w13_sb[:, k, ds(I + off, cw)],
                            start=(k == 0), stop=(k == KT1 - 1),
                        )
                    pgus.append((off, cw, pg, pu))
                del qts[t]
                return pgus

            def fc1_braid():
                """Prologue fc1 for tiles 0 and 1-group0, k-outer so the PE
                consumes w13 k-tiles in DMA arrival order (all 8 PSUM banks)."""
                t0g = []
                for gi, (off, cw) in enumerate(FC1_GROUPS):
                    pg = (ps1 if gi < 2 else ps2).tile(
                        [128, 512], F32, tag=("psg" if gi < 2 else "psa"))
                    pu = (ps1 if gi < 2 else ps2).tile(
                        [128, 512], F32, tag=("psu" if gi < 2 else "psb"))
                    t0g.append((off, cw, pg, pu))
                pgB = ps2.tile([128, 512], F32, tag="psa")
                puB = ps2.tile([128, 512], F32, tag="psb")
                # tile 1's matmuls join the braid 4 steps late (its qt DMA
                # lands after w13 k-tile 5); the last k-tiles finish in rest1
                T1_LAG = 4
                for ki in range(KT1):
                    k = ki
                    for off, cw, pg, pu in t0g:
                        nc.tensor.matmul(
                            pg[:, :cw], qts[0][:, k, :], w13_sb[:, k, ds(off, cw)],
                            start=(k == 0), stop=(k == KT1 - 1),
                        )
                        nc.tensor.matmul(
                            pu[:, :cw], qts[0][:, k, :], w13_sb[:, k, ds(I + off, cw)],
                            start=(k == 0), stop=(k == KT1 - 1),
                        )
                    if ki < T1_LAG:
                        continue
                    kb = ki - T1_LAG
                    off, cw = FC1_GROUPS[0]
                    nc.tensor.matmul(
                        pgB[:, :cw], qts[1][:, kb, :], w13_sb[:, kb, ds(off, cw)],
                        start=(kb == 0), stop=False,
                    )
                    nc.tensor.matmul(
                        puB[:, :cw], qts[1][:, kb, :], w13_sb[:, kb, ds(I + off, cw)],
                        start=(kb == 0), stop=False,
                    )
                del qts[0]
                return t0g, (FC1_GROUPS[0][0], FC1_GROUPS[0][1], pgB, puB), T1_LAG

            def fc1_rest1(g0pair, t1_lag):
                """Tile 1: finish group 0's lagged k-tiles, then groups 1..2."""
                off, cw, pgB, puB = g0pair
                for kb in range(KT1 - t1_lag, KT1):
                    nc.tensor.matmul(
                        pgB[:, :cw], qts[1][:, kb, :], w13_sb[:, kb, ds(off, cw)],
                        start=False, stop=(kb == KT1 - 1),
                    )
                    nc.tensor.matmul(
                        puB[:, :cw], qts[1][:, kb, :], w13_sb[:, kb, ds(I + off, cw)],
                        start=False, stop=(kb == KT1 - 1),
                    )
                pgus = [g0pair]
                for off, cw in FC1_GROUPS[1:]:
                    pg = ps1.tile([128, 512], F32, tag="psg")
                    pu = ps1.tile([128, 512], F32, tag="psu")
                    for k in range(KT1):
                        nc.tensor.matmul(
                            pg[:, :cw], qts[1][:, k, :], w13_sb[:, k, ds(off, cw)],
                            start=(k == 0), stop=(k == KT1 - 1),
                        )
                        nc.tensor.matmul(
                            pu[:, :cw], qts[1][:, k, :], w13_sb[:, k, ds(I + off, cw)],
                            start=(k == 0), stop=(k == KT1 - 1),
                        )
                    pgus.append((off, cw, pg, pu))
                del qts[1]
                return pgus

            def chain(t, pgus):
                """DVE/Act: dequant + SwiGLU + abs-max + requant scales.

                g = (pg * s_in) * s13g ; u = (pu * s_in) * s13u  (reference
                association); act = silu(g) * u; s_a = max|act|/127.
                The s_in tensor_scalar comes first so the PSUM bank frees
                without waiting on the s13 broadcast DMAs.
                """
                sin_col = sin_sb[:, t : t + 1]
                act_t = actp.tile([128, I], F32, tag="act")
                mxs = []
                for gi, (off, cw, pg, pu) in enumerate(pgus):
                    g_sc = gp.tile([128, 512], F32, tag="gsc")
                    nc.vector.tensor_scalar_mul(g_sc[:, :cw], pg[:, :cw], sin_col)
                    nc.vector.tensor_tensor(
                        g_sc[:, :cw], g_sc[:, :cw], s13g_b[:, ds(off, cw)],
                        mybir.AluOpType.mult,
                    )
                    nc.scalar.activation(
                        g_sc[:, :cw], g_sc[:, :cw],
                        mybir.ActivationFunctionType.Silu, bias=zero_b, scale=1.0,
                    )
                    nc.vector.tensor_scalar_mul(
                        act_t[:, ds(off, cw)], pu[:, :cw], sin_col
                    )
                    nc.vector.tensor_tensor(
                        act_t[:, ds(off, cw)], act_t[:, ds(off, cw)],
                        s13u_b[:, ds(off, cw)], mybir.AluOpType.mult,
                    )
                    nc.vector.tensor_tensor(
                        act_t[:, ds(off, cw)], act_t[:, ds(off, cw)], g_sc[:, :cw],
                        mybir.AluOpType.mult,
                    )
                    mx = sp.tile([128, 1], F32, tag=f"mx{gi}")
                    nc.vector.tensor_reduce(
                        mx, act_t[:, ds(off, cw)], axis=mybir.AxisListType.X,
                        op=mybir.AluOpType.max, apply_absolute_value=True,
                    )
                    mxs.append(mx)
                m2 = sp.tile([128, 1], F32, tag="m2")
                nc.vector.tensor_tensor(m2, mxs[0], mxs[1], mybir.AluOpType.max)
                nc.vector.tensor_tensor(m2, m2, mxs[2], mybir.AluOpType.max)
                s_tr = sp.tile([128, 1], F32, tag="s_tr")
                nc.vector.tensor_scalar(
                    s_tr, m2, 1.0 / 127.0, 1e-8,
                    mybir.AluOpType.mult, mybir.AluOpType.max,
                )
                sc_eff = sp.tile([128, 1], F32, tag="sc_eff")  # 1 / s_a
                nc.vector.reciprocal(sc_eff, s_tr)
                c_t = sp.tile([128, 1], F32, tag="c")  # final row scale r * s_a
                nc.vector.tensor_tensor(
                    c_t, s_tr, r_sb[:, t : t + 1], mybir.AluOpType.mult
                )
                return act_t, sc_eff, c_t

            def requant(t, act_t, sc_eff):
                """Act: round(act' * sc_eff) -> bf16 ints; DMA-xbar transpose."""
                act_q = aqp.tile([128, I], BF16, tag="aq")
                for off, cw in RQ_CHUNKS:
                    t1 = rqp.tile([128, 512], F32, tag="rq")
                    nc.scalar.activation(
                        t1[:, :cw], act_t[:, ds(off, cw)],
                        mybir.ActivationFunctionType.Identity,
                        bias=pos223, scale=sc_eff,
                    )
                    nc.scalar.activation(
                        act_q[:, ds(off, cw)], t1[:, :cw],
                        mybir.ActivationFunctionType.Identity, bias=neg223,
                    )
                qa = qatp.tile([128, KT2, 128], BF16, tag="qa")
                nc.sync.dma_start_transpose(qa[:], act_q[:])
                return qa

            def fc2_half(t, qa, op_):
                """PE: one 1024-column half of the second int8 GEMM."""
                pa = ps2.tile([128, 512], F32, tag="psa")
                pb = ps2.tile([128, 512], F32, tag="psb")
                for k in range(KT2):
                    nc.tensor.matmul(
                        pa, qa[:, k, :], w2_sb[:, k, ts(2 * op_, 512)],
                        start=(k == 0), stop=(k == KT2 - 1),
                    )
                    nc.tensor.matmul(
                        pb, qa[:, k, :], w2_sb[:, k, ts(2 * op_ + 1, 512)],
                        start=(k == 0), stop=(k == KT2 - 1),
                    )
                return [(2 * op_, pa), (2 * op_ + 1, pb)]

            def fc2(t, qa):
                """PE: second int8 GEMM."""
                return fc2_half(t, qa, 0) + fc2_half(t, qa, 1)

            def store(t, ppairs, c_t, last=False):
                """DVE: dequant fc2 + per-token gate scale; DMA out."""
                t0 = t * 128
                for oc_i, pp in ppairs:
                    oc = oc_i * 512
                    yc = yp.tile([128, 512], F32, tag="yc")
                    nc.vector.tensor_tensor(
                        yc, pp, s2w_b[:, ds(oc, 512)], mybir.AluOpType.mult
                    )
                    # per-token gate scale on GpSimd (SBUF-only) to keep the
                    # DVE queue clear for the next tile's dequant chain; the
                    # final tile stays on the (then-idle) faster DVE to
                    # shorten the kernel-tail drain
                    if last and oc_i % 2 == 1:
                        nc.vector.tensor_scalar_mul(yc, yc, c_t[:, :])
                    else:
                        nc.gpsimd.tensor_scalar_mul(yc, yc, c_t[:, :])
                    nc.sync.dma_start(x_like(y_d)[t0 : t0 + 128, ds(oc, 512)], yc)

            # ---- software-pipelined tile loop ----
            state = {}
            if nt >= 3:
                # Prologue: braided fc1(0)+fc1(1).g0 (k-outer, all 8 PSUM
                # banks) rides the w13 DMA stream; fc1(2) is front-loaded so
                # the PE stays busy while the w2 stream lands before fc2(0).
                t0g, g0pair, t1_lag = fc1_braid()
                pgus1 = fc1_rest1(g0pair, t1_lag)
                state[0] = chain(0, t0g)
                state[1] = chain(1, pgus1)
                pgus2 = fc1(2)
                act_t, sc_eff, c_t = state.pop(0)
                qa = requant(0, act_t, sc_eff)
                ppairs = fc2(0, qa)
                state[2] = chain(2, pgus2)
                store(0, ppairs, c_t)
                if 3 < nt:
                    prep(3)
                for t in range(1, nt):
                    if t + 2 < nt:
                        pgus_n = fc1(t + 2)
                    act_t, sc_eff, c_t = state.pop(t)
                    qa = requant(t, act_t, sc_eff)
                    if t == nt - 1:
                        # final tile: interleave stores with fc2 so the
                        # kernel-tail drain starts as soon as possible
                        for op_ in range(2):
                            ppairs = fc2_half(t, qa, op_)
                            store(t, ppairs, c_t, last=(op_ == 1))
                        continue
                    ppairs = fc2(t, qa)
                    if t + 2 < nt:
                        state[t + 2] = chain(t + 2, pgus_n)
                    store(t, ppairs, c_t)
                    if t + 3 < nt:
                        prep(t + 3)
            else:
                pgus = fc1(0)
                state[0] = chain(0, pgus)
                for t in range(nt):
                    if t + 1 < nt:
                        pgus_n = fc1(t + 1)
                    act_t, sc_eff, c_t = state.pop(t)
                    qa = requant(t, act_t, sc_eff)
                    ppairs = fc2(t, qa)
                    if t + 1 < nt:
                        state[t + 1] = chain(t + 1, pgus_n)
                    store(t, ppairs, c_t)
                    if t + 2 < nt:
                        prep(t + 2)

    nc.finalize()
    return nc


def x_like(handle):
    """DRamTensorHandle -> AP covering the whole tensor."""
    return handle[:]


def _bcast128(handle, off, n):
    """AP reading handle[off:off+n] replicated across 128 partitions."""
    ap = handle[:][ds(off, n)]
    return bass.AP(tensor=ap.tensor, offset=ap.offset, ap=[[0, 128]] + list(ap.ap))


def _host_prep(hidden_states, gate_weight, w13_weight, w13_weight_scale,
               w2_weight, w2_weight_scale):
    """Host routing + quantization + per-core input maps. -> (in_maps, idxs, C)."""
    x = np.ascontiguousarray(np.asarray(hidden_states, dtype=np.float32))
    gw = np.asarray(gate_weight, dtype=np.float32)
    w13 = np.asarray(w13_weight)
    s13 = np.ascontiguousarray(np.asarray(w13_weight_scale, dtype=np.float32))
    w2 = np.asarray(w2_weight)
    s2w = np.ascontiguousarray(np.asarray(w2_weight_scale, dtype=np.float32))

    # ---- host routing: fp32 gate, softmax, top-2, renormalize ----
    logits = (x @ gw.T).astype(np.float32)
    p = np.exp(logits - logits.max(axis=1, keepdims=True), dtype=np.float32)
    p = (p / p.sum(axis=1, keepdims=True)).astype(np.float32)
    topi = np.argsort(-p, axis=1, kind="stable")[:, :TOPK]  # ties -> lower index
    topv = np.take_along_axis(p, topi, axis=1).astype(np.float32)
    gates = (topv / topv.sum(axis=1, keepdims=True)).astype(np.float32)

    # ---- host dynamic per-token int8 quant (exact reference replica) ----
    s_in = np.maximum(np.abs(x).max(axis=1, keepdims=True) / 127.0, 1e-8)
    s_in = s_in.astype(np.float32)
    q = np.clip(np.round(x / s_in), -127.0, 127.0).astype(np.float32)

    idxs, rvals = [], []
    for e in range(E):
        sel = topi == e
        tok = np.nonzero(sel.any(axis=1))[0]
        r = (gates * sel)[tok].sum(axis=1).astype(np.float32)
        idxs.append(tok)
        rvals.append(r)

    cap = max(128, max(len(t) for t in idxs))
    C = ((cap + 127) // 128) * 128
    nt = C // 128

    q_bf = q.astype(ml_dtypes.bfloat16)
    in_maps = []
    for e in range(E):
        n_e = len(idxs[e])
        qg = np.zeros((C, H), dtype=ml_dtypes.bfloat16)
        qg[:n_e] = q_bf[idxs[e]]
        # qt[t, p, k, j] = qg[t*128 + j, 128*k + p]
        qt = np.ascontiguousarray(
            qg.T.reshape(KT1, 128, nt, 128).transpose(2, 1, 0, 3)
        )
        sin_flat = np.zeros(C, dtype=np.float32)
        sin_flat[:n_e] = s_in[idxs[e], 0]
        sin_pack = np.ascontiguousarray(sin_flat.reshape(nt, 128).T)
        r_flat = np.zeros(C, dtype=np.float32)
        r_flat[:n_e] = rvals[e]
        r_pack = np.ascontiguousarray(r_flat.reshape(nt, 128).T)
        w13t = np.ascontiguousarray(w13[e].T).reshape(KT1, 128, 2 * I)
        w2t = np.ascontiguousarray(w2[e].T).reshape(KT2, 128, H)
        in_maps.append({
            "qt": qt,
            "sin": sin_pack,
            "r": r_pack,
            "w13t": w13t,
            "w2t": w2t,
            "s13": np.ascontiguousarray(s13[e]),
            "s2w": np.ascontiguousarray(s2w[e]),
        })
    return in_maps, idxs, C


def kernel(hidden_states, gate_weight, w13_weight, w13_weight_scale,
           w2_weight, w2_weight_scale):
    in_maps, idxs, C = _host_prep(hidden_states, gate_weight, w13_weight,
                                  w13_weight_scale, w2_weight, w2_weight_scale)
    if C not in _cache:
        _cache[C] = _build_program(C)
    nc = _cache[C]

    trace = bool(int(os.environ.get("MOE_TRACE", "0")))
    br = run_bass_kernel_spmd(nc, in_maps, list(range(E)), trace=trace)
    global LAST_EXEC_NS
    LAST_EXEC_NS = br.exec_time_ns
    res = br.results

    out = np.zeros((T, H), dtype=np.float32)
    for e in range(E):
        n_e = len(idxs[e])
        if n_e:
            out[idxs[e]] += np.asarray(res[e]["y"])[:n_e]
    return out


# revision 63
# speedup vs baseline: 1.0100x; 1.0014x over previous
"""Trainium2 Bass kernel for nn_IxformerQuantMoe (quantized top-2 MoE, E=8 experts).

Strategy (expert-parallel across 8 NeuronCores):
  - Host computes the fp32 gate (softmax + top-2 + renormalize), the per-token
    dynamic int8 quantization of hidden_states (exact numpy replica of the
    reference), and routes tokens: for each expert e, gathers the quantized
    rows of the tokens whose top-2 contains e, padded to a common capacity C
    (multiple of 128), pre-transposed to the contraction-major tile layout
    the PE array consumes.
  - Core e runs the expert FFN for its token set:
      int8 GEMM fc1 (exact in bf16, fp32 PSUM) -> dequant+SwiGLU -> dynamic
      requant (round-to-nearest via the 1.5*2^23 trick) -> DMA-xbar transpose
      -> int8 GEMM fc2 -> dequant + gate scaling.
  - Throughput structure (CoreSim: PE ~95% busy):
      * per-tile work is software-pipelined one tile ahead (fc1 of tile t+1
        runs while tile t requants/fc2s) so the PE never waits on the
        dequant/requant chain;
      * weights ship as int8 (half the HBM bytes) and are up-converted to
        bf16 on-chip on the otherwise-idle DVE/GpSimd engines;
      * the prologue runs fc1(0) k-OUTER braided with fc1(1).group0 across
        all 8 PSUM banks so the PE consumes w13 k-tiles in DMA arrival
        order, and fc1(2) is front-loaded before fc2(0) to cover the w2
        stream;
      * the dequant applies (psum * s_in) first (reference association),
        which also frees PSUM banks without waiting on the s13 scale DMAs.
  - Host scatter-adds each expert's output rows into the final [T, H] output
    (the weighted top-2 combine).
"""

import os
import sys

for _p in ("/opt/trn_rl_repo", "/root/.axon_site/_ro/trn_rl_repo"):
    if os.path.isdir(_p) and _p not in sys.path:
        sys.path.insert(0, _p)

import numpy as np
import ml_dtypes

import concourse.bass as bass
import concourse.bacc as bacc
import concourse.tile as tile
from concourse import mybir
from concourse.bass import ds, ts
from concourse.bass_utils import run_bass_kernel_spmd

T, H, I, E, TOPK = 4096, 2048, 1408, 8, 2
KT1 = H // 128     # 16 k-tiles for fc1 contraction
KT2 = I // 128     # 11 k-tiles for fc2 contraction
TWO23 = 12582912.0  # 1.5*2^23: fp32 add/sub rounds to nearest integer (RNE)

F32 = mybir.dt.float32
BF16 = mybir.dt.bfloat16
I8 = mybir.dt.int8

FC1_GROUPS = [(0, 512), (512, 512), (1024, 384)]   # column groups over I
RQ_CHUNKS = [(0, 512), (512, 512), (1024, 384)]    # requant chunks over I

_cache = {}
LAST_EXEC_NS = None


def _build_program(C, tail_rows=128):
    """Bass program run identically (SPMD) on 8 cores; per-core data differs.

    tail_rows: number of real token rows in the last tile across all cores —
    the last tile's stores only cover these rows (the rest stay zero in the
    donated output buffer).
    """
    nt = C // 128
    nc = bacc.Bacc(None, target_bir_lowering=False)

    qt_d = nc.declare_dram_parameter("qt", [nt, 128, KT1, 128], BF16, isOutput=False)
    sin_d = nc.declare_dram_parameter("sin", [128, nt], F32, isOutput=False)
    r_d = nc.declare_dram_parameter("r", [128, nt], F32, isOutput=False)
    w13_d = nc.declare_dram_parameter("w13t", [KT1, 128, 2 * I], I8, isOutput=False)
    w2_d = nc.declare_dram_parameter("w2t", [KT2, 128, H], I8, isOutput=False)
    s13_d = nc.declare_dram_parameter("s13", [2 * I], F32, isOutput=False)
    s2w_d = nc.declare_dram_parameter("s2w", [H], F32, isOutput=False)
    y_d = nc.declare_dram_parameter("y", [C, H], F32, isOutput=True)

    with tile.TileContext(nc) as tc:
        with (
            tc.tile_pool(name="singles", bufs=1) as singles,
            tc.tile_pool(name="qtp", bufs=2) as qtp,
            tc.tile_pool(name="gp", bufs=2) as gp,
            tc.tile_pool(name="actp", bufs=2) as actp,
            tc.tile_pool(name="aqp", bufs=2) as aqp,
            tc.tile_pool(name="rqp", bufs=2) as rqp,
            tc.tile_pool(name="qatp", bufs=2) as qatp,
            tc.tile_pool(name="yp", bufs=2) as yp,
            tc.tile_pool(name="wsp", bufs=2) as wsp,
            tc.tile_pool(name="sp", bufs=3) as sp,
            tc.tile_pool(name="ps1", bufs=2, space="PSUM") as ps1,
            tc.tile_pool(name="ps2", bufs=2, space="PSUM") as ps2,
        ):
            # ---- resident tiles ----
            w13_sb = singles.tile([128, KT1, 2 * I], BF16)
            w2_sb = singles.tile([128, KT2, H], BF16)
            s13g_b = singles.tile([128, I], F32)
            s13u_b = singles.tile([128, I], F32)
            s2w_b = singles.tile([128, H], F32)
            sin_sb = singles.tile([128, nt], F32)
            r_sb = singles.tile([128, nt], F32)

            neg223 = singles.tile([128, 1], F32)
            nc.vector.memset(neg223, -TWO23)
            pos223 = singles.tile([128, 1], F32)
            nc.vector.memset(pos223, TWO23)
            zero_b = singles.tile([128, 1], F32)
            nc.vector.memset(zero_b, 0.0)

            qts = {}

            def prep(t, split=False):
                """DMA the pre-transposed quantized input tile for token-tile t."""
                qt = qtp.tile([128, KT1, 128], BF16, tag="qt")
                if split:
                    # two halves so the braid's k=0 matmuls start sooner
                    half = KT1 // 2
                    nc.sync.dma_start(qt[:, :half, :], x_like(qt_d)[t][:, :half, :])
                    nc.sync.dma_start(qt[:, half:, :], x_like(qt_d)[t][:, half:, :])
                else:
                    nc.sync.dma_start(qt, x_like(qt_d)[t])
                qts[t] = qt

            # ---- prologue DMA order: int8 weights streamed and up-converted
            # on-chip (w13 alternating DVE/Act, w2 on GpSimd) so fc1(0) can
            # start as soon as w13 k-tile 0 lands.
            # warm the PE p-state/HAM during the initial DMA wait with dummy
            # matmuls on (junk) SBUF; the result is never read
            wmup = ps2.tile([128, 512], F32, tag="psa")
            for _ in range(9):
                nc.tensor.matmul(
                    wmup[:1, :], w13_sb[:, 0, 0:1], w13_sb[:, 1, 0:512],
                    start=True, stop=True,
                )

            # k0 specially split (and interleaved with qt(0)'s halves) so the
            # braid's first g-side matmuls start as early as possible
            qt0 = qtp.tile([128, KT1, 128], BF16, tag="qt")
            qts[0] = qt0
            # w13-k0's g-half goes first: its DMA->convert chain is longer
            # than qt(0)'s plain DMA, so it gates the first matmul
            ws0 = wsp.tile([128, 2 * I], I8, tag="wse")
            nc.sync.dma_start(ws0[:, :I], x_like(w13_d)[0][:, :I])
            nc.vector.tensor_copy(w13_sb[:, 0, :I], ws0[:, :I])
            nc.sync.dma_start(qt0[:, : KT1 // 2, :], x_like(qt_d)[0][:, : KT1 // 2, :])
            nc.sync.dma_start(ws0[:, I:], x_like(w13_d)[0][:, I:])
            nc.vector.tensor_copy(w13_sb[:, 0, I:], ws0[:, I:])
            nc.sync.dma_start(qt0[:, KT1 // 2 :, :], x_like(qt_d)[0][:, KT1 // 2 :, :])
            for k in range(1, KT1):
                ws = wsp.tile([128, 2 * I], I8, tag=("wse" if k % 2 == 0 else "wso"))
                nc.sync.dma_start(ws, x_like(w13_d)[k])
                # the braid-critical early k-tiles convert on the faster DVE
                # (g/u halves, since the braid's pg matmuls only need g);
                # odd tiles from k7 go to GpSimd, which has more slack
                if k <= 3:
                    nc.vector.tensor_copy(w13_sb[:, k, :I], ws[:, :I])
                    nc.vector.tensor_copy(w13_sb[:, k, I:], ws[:, I:])
                elif k == 5 or k % 2 == 0:
                    nc.vector.tensor_copy(w13_sb[:, k, :], ws)
                else:
                    nc.gpsimd.tensor_copy(w13_sb[:, k, :], ws)
                if k == 5 and nt > 1:
                    prep(1)
            nc.sync.dma_start(sin_sb, x_like(sin_d))
            nc.sync.dma_start(r_sb, x_like(r_d))
            nc.sync.dma_start(s13g_b, _bcast128(s13_d, 0, I))
            nc.sync.dma_start(s13u_b, _bcast128(s13_d, I, I))
            if nt > 2:
                prep(2)
            for k in range(KT2):
                ws = wsp.tile([128, 2 * I], I8, tag=("wse" if k % 2 == 0 else "wso"))
                nc.sync.dma_start(ws[:, :H], x_like(w2_d)[k])
                nc.gpsimd.tensor_copy(w2_sb[:, k, :], ws[:, :H])
            nc.sync.dma_start(s2w_b, _bcast128(s2w_d, 0, H))

            def fc1(t):
                """PE: int8-as-bf16 GEMM into PSUM group pairs."""
                pgus = []
                for off, cw in FC1_GROUPS:
                    pg = ps1.tile([128, 512], F32, tag="psg")
                    pu = ps1.tile([128, 512], F32, tag="psu")
                    for k in range(KT1):
                        nc.tensor.matmul(
                            pg[:, :cw], qts[t][:, k, :], w13_sb[:, k, ds(off, cw)],
                            start=(k == 0), stop=(k == KT1 - 1),
                        )
                        nc.tensor.matmul(
                            pu[:, :cw], qts[t][:, k, :], w13_sb[:, k, ds(I + off, cw)],
                            start=(k == 0), stop=(k == KT1 - 1),
                        )
                    pgus.append((off, cw, pg, pu))
                del qts[t]
                return pgus

            def fc1_braid():
                """Prologue fc1 for tiles 0 and 1-group0, k-outer so the PE
                consumes w13 k-tiles in DMA arrival order (all 8 PSUM banks)."""
                t0g = []
                for gi, (off, cw) in enumerate(FC1_GROUPS):
                    pg = (ps1 if gi < 2 else ps2).tile(
                        [128, 512], F32, tag=("psg" if gi < 2 else "psa"))
                    pu = (ps1 if gi < 2 else ps2).tile(
                        [128, 512], F32, tag=("psu" if gi < 2 else "psb"))
                    t0g.append((off, cw, pg, pu))
                pgB = ps2.tile([128, 512], F32, tag="psa")
                puB = ps2.tile([128, 512], F32, tag="psb")
                # tile 1's matmuls join the braid 4 steps late (its qt DMA
                # lands after w13 k-tile 5); the last k-tiles finish in rest1
                T1_LAG = 4
                for ki in range(KT1):
                    k = ki
                    for off, cw, pg, pu in t0g:
                        nc.tensor.matmul(
                            pg[:, :cw], qts[0][:, k, :], w13_sb[:, k, ds(off, cw)],
                            start=(k == 0), stop=(k == KT1 - 1),
                        )
                        nc.tensor.matmul(
                            pu[:, :cw], qts[0][:, k, :], w13_sb[:, k, ds(I + off, cw)],
                            start=(k == 0), stop=(k == KT1 - 1),
                        )
                    if ki < T1_LAG:
                        continue
                    kb = ki - T1_LAG
                    off, cw = FC1_GROUPS[0]
                    nc.tensor.matmul(
                        pgB[:, :cw], qts[1][:, kb, :], w13_sb[:, kb, ds(off, cw)],
                        start=(kb == 0), stop=False,
                    )
                    nc.tensor.matmul(
                        puB[:, :cw], qts[1][:, kb, :], w13_sb[:, kb, ds(I + off, cw)],
                        start=(kb == 0), stop=False,
                    )
                del qts[0]
                return t0g, (FC1_GROUPS[0][0], FC1_GROUPS[0][1], pgB, puB), T1_LAG

            def fc1_rest1(g0pair, t1_lag):
                """Tile 1: finish group 0's lagged k-tiles, then groups 1..2."""
                off, cw, pgB, puB = g0pair
                for kb in range(KT1 - t1_lag, KT1):
                    nc.tensor.matmul(
                        pgB[:, :cw], qts[1][:, kb, :], w13_sb[:, kb, ds(off, cw)],
                        start=False, stop=(kb == KT1 - 1),
                    )
                    nc.tensor.matmul(
                        puB[:, :cw], qts[1][:, kb, :], w13_sb[:, kb, ds(I + off, cw)],
                        start=False, stop=(kb == KT1 - 1),
                    )
                pgus = [g0pair]
                for off, cw in FC1_GROUPS[1:]:
                    pg = ps1.tile([128, 512], F32, tag="psg")
                    pu = ps1.tile([128, 512], F32, tag="psu")
                    for k in range(KT1):
                        nc.tensor.matmul(
                            pg[:, :cw], qts[1][:, k, :], w13_sb[:, k, ds(off, cw)],
                            start=(k == 0), stop=(k == KT1 - 1),
                        )
                        nc.tensor.matmul(
                            pu[:, :cw], qts[1][:, k, :], w13_sb[:, k, ds(I + off, cw)],
                            start=(k == 0), stop=(k == KT1 - 1),
                        )
                    pgus.append((off, cw, pg, pu))
                del qts[1]
                return pgus

            def chain(t, pgus):
                """DVE/Act: dequant + SwiGLU + abs-max + requant scales.

                g = (pg * s_in) * s13g ; u = (pu * s_in) * s13u  (reference
                association); act = silu(g) * u; s_a = max|act|/127.
                The s_in tensor_scalar comes first so the PSUM bank frees
                without waiting on the s13 broadcast DMAs.
                """
                sin_col = sin_sb[:, t : t + 1]
                act_t = actp.tile([128, I], F32, tag="act")
                mxs = []
                for gi, (off, cw, pg, pu) in enumerate(pgus):
                    g_sc = gp.tile([128, 512], F32, tag="gsc")
                    nc.vector.tensor_scalar_mul(g_sc[:, :cw], pg[:, :cw], sin_col)
                    nc.vector.tensor_tensor(
                        g_sc[:, :cw], g_sc[:, :cw], s13g_b[:, ds(off, cw)],
                        mybir.AluOpType.mult,
                    )
                    nc.scalar.activation(
                        g_sc[:, :cw], g_sc[:, :cw],
                        mybir.ActivationFunctionType.Silu, bias=zero_b, scale=1.0,
                    )
                    nc.vector.tensor_scalar_mul(
                        act_t[:, ds(off, cw)], pu[:, :cw], sin_col
                    )
                    nc.vector.tensor_tensor(
                        act_t[:, ds(off, cw)], act_t[:, ds(off, cw)],
                        s13u_b[:, ds(off, cw)], mybir.AluOpType.mult,
                    )
                    nc.vector.tensor_tensor(
                        act_t[:, ds(off, cw)], act_t[:, ds(off, cw)], g_sc[:, :cw],
                        mybir.AluOpType.mult,
                    )
                    mx = sp.tile([128, 1], F32, tag=f"mx{gi}")
                    nc.vector.tensor_reduce(
                        mx, act_t[:, ds(off, cw)], axis=mybir.AxisListType.X,
                        op=mybir.AluOpType.max, apply_absolute_value=True,
                    )
                    mxs.append(mx)
                m2 = sp.tile([128, 1], F32, tag="m2")
                nc.vector.tensor_tensor(m2, mxs[0], mxs[1], mybir.AluOpType.max)
                nc.vector.tensor_tensor(m2, m2, mxs[2], mybir.AluOpType.max)
                s_tr = sp.tile([128, 1], F32, tag="s_tr")
                nc.vector.tensor_scalar(
                    s_tr, m2, 1.0 / 127.0, 1e-8,
                    mybir.AluOpType.mult, mybir.AluOpType.max,
                )
                sc_eff = sp.tile([128, 1], F32, tag="sc_eff")  # 1 / s_a
                nc.vector.reciprocal(sc_eff, s_tr)
                c_t = sp.tile([128, 1], F32, tag="c")  # final row scale r * s_a
                nc.vector.tensor_tensor(
                    c_t, s_tr, r_sb[:, t : t + 1], mybir.AluOpType.mult
                )
                return act_t, sc_eff, c_t

            def requant(t, act_t, sc_eff):
                """Act: round(act' * sc_eff) -> bf16 ints; DMA-xbar transpose."""
                act_q = aqp.tile([128, I], BF16, tag="aq")
                for off, cw in RQ_CHUNKS:
                    t1 = rqp.tile([128, 512], F32, tag="rq")
                    nc.scalar.activation(
                        t1[:, :cw], act_t[:, ds(off, cw)],
                        mybir.ActivationFunctionType.Identity,
                        bias=pos223, scale=sc_eff,
                    )
                    nc.scalar.activation(
                        act_q[:, ds(off, cw)], t1[:, :cw],
                        mybir.ActivationFunctionType.Identity, bias=neg223,
                    )
                qa = qatp.tile([128, KT2, 128], BF16, tag="qa")
                nc.sync.dma_start_transpose(qa[:], act_q[:])
                return qa

            def fc2_half(t, qa, op_):
                """PE: one 1024-column half of the second int8 GEMM."""
                pa = ps2.tile([128, 512], F32, tag="psa")
                pb = ps2.tile([128, 512], F32, tag="psb")
                for k in range(KT2):
                    nc.tensor.matmul(
                        pa, qa[:, k, :], w2_sb[:, k, ts(2 * op_, 512)],
                        start=(k == 0), stop=(k == KT2 - 1),
                    )
                    nc.tensor.matmul(
                        pb, qa[:, k, :], w2_sb[:, k, ts(2 * op_ + 1, 512)],
                        start=(k == 0), stop=(k == KT2 - 1),
                    )
                return [(2 * op_, pa), (2 * op_ + 1, pb)]

            def fc2(t, qa):
                """PE: second int8 GEMM."""
                return fc2_half(t, qa, 0) + fc2_half(t, qa, 1)

            def store(t, ppairs, c_t, last=False, rows=128):
                """DVE: dequant fc2 + per-token gate scale; DMA out."""
                t0 = t * 128
                for oc_i, pp in ppairs:
                    oc = oc_i * 512
                    yc = yp.tile([128, 512], F32, tag="yc")
                    nc.vector.tensor_tensor(
                        yc[:rows, :], pp[:rows, :], s2w_b[:rows, ds(oc, 512)],
                        mybir.AluOpType.mult,
                    )
                    # per-token gate scale on GpSimd (SBUF-only) to keep the
                    # DVE queue clear for the next tile's dequant chain; the
                    # final tile splits across DVE too (both then idle) to
                    # shorten the kernel-tail drain
                    if last and oc_i % 2 == 1:
                        nc.vector.tensor_scalar_mul(
                            yc[:rows, :], yc[:rows, :], c_t[:rows, :]
                        )
                    else:
                        nc.gpsimd.tensor_scalar_mul(
                            yc[:rows, :], yc[:rows, :], c_t[:rows, :]
                        )
                    nc.sync.dma_start(
                        x_like(y_d)[t0 : t0 + rows, ds(oc, 512)], yc[:rows, :]
                    )

            # ---- software-pipelined tile loop ----
            state = {}
            if nt >= 3:
                # Prologue: braided fc1(0)+fc1(1).g0 (k-outer, all 8 PSUM
                # banks) rides the w13 DMA stream; fc1(2) is front-loaded so
                # the PE stays busy while the w2 stream lands before fc2(0).
                t0g, g0pair, t1_lag = fc1_braid()
                pgus1 = fc1_rest1(g0pair, t1_lag)
                state[0] = chain(0, t0g)
                state[1] = chain(1, pgus1)
                pgus2 = fc1(2)
                act_t, sc_eff, c_t = state.pop(0)
                qa = requant(0, act_t, sc_eff)
                ppairs = fc2(0, qa)
                state[2] = chain(2, pgus2)
                store(0, ppairs, c_t)
                if 3 < nt:
                    prep(3)
                for t in range(1, nt):
                    if t + 2 < nt:
                        pgus_n = fc1(t + 2)
                    act_t, sc_eff, c_t = state.pop(t)
                    qa = requant(t, act_t, sc_eff)
                    if t == nt - 1:
                        # final tile: interleave stores with fc2 so the
                        # kernel-tail drain starts as soon as possible
                        for op_ in range(2):
                            ppairs = fc2_half(t, qa, op_)
                            store(t, ppairs, c_t, last=(op_ == 1), rows=tail_rows)
                        continue
                    ppairs = fc2(t, qa)
                    if t + 2 < nt:
                        state[t + 2] = chain(t + 2, pgus_n)
                    store(t, ppairs, c_t)
                    if t + 3 < nt:
                        prep(t + 3)
            else:
                pgus = fc1(0)
                state[0] = chain(0, pgus)
                for t in range(nt):
                    if t + 1 < nt:
                        pgus_n = fc1(t + 1)
                    act_t, sc_eff, c_t = state.pop(t)
                    qa = requant(t, act_t, sc_eff)
                    ppairs = fc2(t, qa)
                    if t + 1 < nt:
                        state[t + 1] = chain(t + 1, pgus_n)
                    store(t, ppairs, c_t)
                    if t + 2 < nt:
                        prep(t + 2)

    nc.finalize()
    return nc


def x_like(handle):
    """DRamTensorHandle -> AP covering the whole tensor."""
    return handle[:]


def _bcast128(handle, off, n):
    """AP reading handle[off:off+n] replicated across 128 partitions."""
    ap = handle[:][ds(off, n)]
    return bass.AP(tensor=ap.tensor, offset=ap.offset, ap=[[0, 128]] + list(ap.ap))


def _host_prep(hidden_states, gate_weight, w13_weight, w13_weight_scale,
               w2_weight, w2_weight_scale):
    """Host routing + quantization + per-core input maps. -> (in_maps, idxs, C)."""
    x = np.ascontiguousarray(np.asarray(hidden_states, dtype=np.float32))
    gw = np.asarray(gate_weight, dtype=np.float32)
    w13 = np.asarray(w13_weight)
    s13 = np.ascontiguousarray(np.asarray(w13_weight_scale, dtype=np.float32))
    w2 = np.asarray(w2_weight)
    s2w = np.ascontiguousarray(np.asarray(w2_weight_scale, dtype=np.float32))

    # ---- host routing: fp32 gate, softmax, top-2, renormalize ----
    logits = (x @ gw.T).astype(np.float32)
    p = np.exp(logits - logits.max(axis=1, keepdims=True), dtype=np.float32)
    p = (p / p.sum(axis=1, keepdims=True)).astype(np.float32)
    topi = np.argsort(-p, axis=1, kind="stable")[:, :TOPK]  # ties -> lower index
    topv = np.take_along_axis(p, topi, axis=1).astype(np.float32)
    gates = (topv / topv.sum(axis=1, keepdims=True)).astype(np.float32)

    # ---- host dynamic per-token int8 quant (exact reference replica) ----
    s_in = np.maximum(np.abs(x).max(axis=1, keepdims=True) / 127.0, 1e-8)
    s_in = s_in.astype(np.float32)
    q = np.clip(np.round(x / s_in), -127.0, 127.0).astype(np.float32)

    idxs, rvals = [], []
    for e in range(E):
        sel = topi == e
        tok = np.nonzero(sel.any(axis=1))[0]
        r = (gates * sel)[tok].sum(axis=1).astype(np.float32)
        idxs.append(tok)
        rvals.append(r)

    cap = max(128, max(len(t) for t in idxs))
    C = ((cap + 127) // 128) * 128
    nt = C // 128

    q_bf = q.astype(ml_dtypes.bfloat16)
    in_maps = []
    for e in range(E):
        n_e = len(idxs[e])
        qg = np.zeros((C, H), dtype=ml_dtypes.bfloat16)
        qg[:n_e] = q_bf[idxs[e]]
        # qt[t, p, k, j] = qg[t*128 + j, 128*k + p]
        qt = np.ascontiguousarray(
            qg.T.reshape(KT1, 128, nt, 128).transpose(2, 1, 0, 3)
        )
        sin_flat = np.zeros(C, dtype=np.float32)
        sin_flat[:n_e] = s_in[idxs[e], 0]
        sin_pack = np.ascontiguousarray(sin_flat.reshape(nt, 128).T)
        r_flat = np.zeros(C, dtype=np.float32)
        r_flat[:n_e] = rvals[e]
        r_pack = np.ascontiguousarray(r_flat.reshape(nt, 128).T)
        w13t = np.ascontiguousarray(w13[e].T).reshape(KT1, 128, 2 * I)
        w2t = np.ascontiguousarray(w2[e].T).reshape(KT2, 128, H)
        in_maps.append({
            "qt": qt,
            "sin": sin_pack,
            "r": r_pack,
            "w13t": w13t,
            "w2t": w2t,
            "s13": np.ascontiguousarray(s13[e]),
            "s2w": np.ascontiguousarray(s2w[e]),
        })
    return in_maps, idxs, C


def kernel(hidden_states, gate_weight, w13_weight, w13_weight_scale,
           w2_weight, w2_weight_scale):
    in_maps, idxs, C = _host_prep(hidden_states, gate_weight, w13_weight,
                                  w13_weight_scale, w2_weight, w2_weight_scale)
    tail_rows = max(len(i) for i in idxs) - (C // 128 - 1) * 128
    key = (C, tail_rows)
    if key not in _cache:
        _cache[key] = _build_program(C, tail_rows)
    nc = _cache[key]

    trace = bool(int(os.environ.get("MOE_TRACE", "0")))
    br = run_bass_kernel_spmd(nc, in_maps, list(range(E)), trace=trace)
    global LAST_EXEC_NS
    LAST_EXEC_NS = br.exec_time_ns
    res = br.results

    out = np.zeros((T, H), dtype=np.float32)
    for e in range(E):
        n_e = len(idxs[e])
        if n_e:
            out[idxs[e]] += np.asarray(res[e]["y"])[:n_e]
    return out
